# revision 29
# baseline (speedup 1.0000x reference)
"""Trainium2 Bass kernel for ViTDet-style attention with decomposed
relative-position bias.

Problem shapes (hardcoded):
  x: (4, 32, 32, 768) f32, Wqkv: (768, 2304), Wproj: (768, 768),
  bproj: (768,), rel_pos_h/w: (63, 64).
  12 heads, head_dim 64, S = 32*32 = 1024.

Sharding: 48 (batch, head) pairs -> 6 heads per core, all of one batch per
core-pair. Each core computes its heads' attention and a partial output
projection (its heads' channel rows of Wproj); the host sums the two
partials (bf16) per batch and adds bproj.

Device algorithm per core (bf16 matmuls, fp32 PSUM accumulation):
  - inputs land in a handful of large strided DMAs (the sync engine's
    per-DMA issue cost, ~0.7us, was serializing the load phase)
  - qkT = Wqk^T @ x^T  (x^T supplied pre-transposed by host; k pre-scaled)
  - v   = x @ Wv       (natural layout, with an appended ones column)
  - rel-pos bias computed directly per h/w block with the rel table slice
    as stationary and a 3D strided AP moving over all 6 heads at once;
    the w-axis is staged w-major (contiguous copies) and permuted once
    per head.
  - scoresT (k x q) = kaugT^T @ qaugT in ONE K=128 matmul per tile:
    aug rows 0-63 = kT / qT, 64-95 = one-hot(h) / BhT, 96-127 = one-hot(w)/BwT
    (one-hot rows DMA'd straight from DRAM into kaug).
  - eT = exp(scoresT): head A of each pair on ScalarE (table exp), head B
    on VectorE via the Schraudolph bit trick (i16 = round(a*x+b) viewed as
    bf16) so the two exps of a round run on different engines.
  - avT (65 x q) accumulates v_aug^T-matmul over k blocks; row 64 = softmax
    denominator via the ones column.  Head pairs are interleaved with av
    matmuls one k-block behind the score matmuls.
  - normalize on-chip: copy av to SBUF (frees PSUM fast), SBUF->SBUF DMA
    reshapes the denominator row to (128,8), DVE reciprocal + cast to bf16,
    DMA back to a row, gpsimd partition-broadcast (last pair: PE K=1
    broadcast, since the PE is idle there), DVE multiply.
  - partial = out_heads @ Wproj_shard, emitted bf16 to DRAM.
"""

import numpy as np

import concourse.bass as bass
import concourse.bacc as bacc
import concourse.mybir as mybir
import concourse.tile as tile
from concourse.bass_utils import run_bass_kernel_spmd

F32 = mybir.dt.float32
BF16 = mybir.dt.bfloat16
I16 = mybir.dt.int16

NH = 12          # total heads
C = 768
HD = 64
H = W = 32
S = H * W        # 1024
B = 4
NCORES = 8
HPC = NH * B // NCORES   # heads per core = 6
NCH = 6                  # C // 128 input-channel chunks
NKB = S // 128           # 8 k blocks
NQB = S // 128           # 8 q blocks
NHALF = 512              # matmul moving-dim half

# Schraudolph exp constants for bf16 output: bits = round(EXP_A*x + EXP_B)
EXP_A = 184.66502304     # 2^7 / ln 2
EXP_B = 16247.75


def build_program():
    nc = bacc.Bacc("TRN2", target_bir_lowering=False, debug=False)

    # inputs pre-chunked on the host so every DMA is a contiguous DRAM read
    xT = nc.declare_dram_parameter("xT", [128, NCH * S], BF16, isOutput=False)
    wqk = nc.declare_dram_parameter("wqk", [128, NCH * 2 * HPC * HD], BF16,
                                    isOutput=False)
    wv = nc.declare_dram_parameter("wv", [128, NCH * HPC * HD], BF16,
                                   isOutput=False)
    wproj = nc.declare_dram_parameter("wproj", [128, 3 * C], BF16, isOutput=False)
    rhT = nc.declare_dram_parameter("rhT", [HD, 2 * H - 1], BF16, isOutput=False)
    rwT = nc.declare_dram_parameter("rwT", [HD, 2 * W - 1], BF16, isOutput=False)
    onehot = nc.declare_dram_parameter("onehot", [65, S], BF16, isOutput=False)
    oh6 = nc.declare_dram_parameter("oh6", [64, HPC * S], BF16, isOutput=False)
    out = nc.declare_dram_parameter("out", [S, C], BF16, isOutput=True)

    with tile.TileContext(nc) as tc:
        with (
            tc.tile_pool(name="persist", bufs=1) as persist,
            tc.tile_pool(name="pa", bufs=2, space="PSUM") as pa,
            tc.tile_pool(name="pb", bufs=2, space="PSUM") as pb,
            tc.tile_pool(name="et", bufs=4) as et_pool,
            tc.tile_pool(name="small", bufs=2) as small,
        ):
            # all-heads augmented k/q tiles (128, 6*S)
            kaug = persist.tile([128, HPC * S], BF16, tag="kaug", name="kaug")
            qaug = persist.tile([128, HPC * S], BF16, tag="qaug", name="qaug")
            ones_sb = persist.tile([1, 64], BF16, tag="ones", name="ones_sb")
            nc.vector.memset(ones_sb[:], 1.0)

            # ---- persistent SBUF loads: few big strided DMAs ----
            xT_all = persist.tile([128, NCH * S], BF16, tag="xT", name="xT_all")
            wv_all = persist.tile([128, NCH * HPC * HD], BF16, tag="wv", name="wv_all")
            wqk_all = persist.tile([128, NCH * 2 * HPC * HD], BF16, tag="wqk",
                                   name="wqk_all")
            wproj_all = persist.tile([128, 3 * C], BF16, tag="wproj",
                                     name="wproj_all")
            # spread the loads across per-engine DMA queues so the
            # transfers run in parallel instead of serializing on one queue
            nc.sync.dma_start(xT_all[:, 0:3 * S], xT[:, 0:3 * S])
            nc.sync.dma_start(xT_all[:, 3 * S:6 * S], xT[:, 3 * S:6 * S])
            nc.scalar.dma_start(wv_all[:], wv[:, :])
            nc.scalar.dma_start(wqk_all[:, 0:3 * 768], wqk[:, 0:3 * 768])
            nc.gpsimd.dma_start(wqk_all[:, 3 * 768:], wqk[:, 3 * 768:])
            oh = persist.tile([65, S], BF16, tag="onehot", name="onehot")
            nc.sync.dma_start(oh[:], onehot[:, :])
            rhT_sb = persist.tile([HD, 2 * H - 1], BF16, tag="rhT", name="rhT_sb")
            nc.sync.dma_start(rhT_sb[:], rhT[:, :])
            rwT_sb = persist.tile([HD, 2 * W - 1], BF16, tag="rwT", name="rwT_sb")
            nc.sync.dma_start(rwT_sb[:], rwT[:, :])
            # off the critical path: issue from the (idle) gpsimd queue
            nc.gpsimd.dma_start(wproj_all[:], wproj[:, :])
            nc.gpsimd.dma_start(kaug[64:128, :], oh6[:, :])

            def xs(ci):
                return xT_all[:, S * ci:S * (ci + 1)]

            # ---- v projection (natural) + ones column ----
            # v_sb[sb]: (128, 6*65) cols [65i..65i+64) = head i v, col 65i+64 = 1
            v_sb = [persist.tile([128, HPC * (HD + 1)], BF16, tag=f"v{sb}", name=f"v{sb}")
                    for sb in range(NKB)]
            for sb in range(NKB):
                vp = pa.tile([128, HPC * HD + HPC], F32, tag="big", name="vp")
                for ci in range(NCH):
                    nc.tensor.matmul(
                        vp[:, 0:HPC * HD],
                        xs(ci)[:, 128 * sb:128 * (sb + 1)],
                        wv_all[:, HPC * HD * ci:HPC * HD * (ci + 1)],
                        start=(ci == 0), stop=(ci == NCH - 1))
                nc.tensor.matmul(vp[:, HPC * HD:HPC * HD + HPC],
                                 oh[64:65, 128 * sb:128 * (sb + 1)],
                                 oh[64:65, 0:HPC], start=True, stop=True)
                vdst = v_sb[sb].rearrange("p (i c) -> p i c", c=HD + 1)
                nc.vector.tensor_copy(
                    vdst[:, :, 0:HD],
                    vp[:, 0:HPC * HD].rearrange("p (i c) -> p i c", c=HD))
                nc.vector.tensor_copy(
                    vdst[:, :, HD:HD + 1],
                    vp[:, HPC * HD:HPC * HD + HPC].rearrange("p (i c) -> p i c", c=1))

            # ---- qk projection (transposed layout) ----
            # octile t covers oc rows [128t, 128t+128): t<3 -> q, t>=3 -> k
            def qk_octile(t):
                qp = pa.tile([128, S], F32, tag="big", name="qp")
                for ci in range(NCH):
                    for nh in range(S // NHALF):
                        nc.tensor.matmul(
                            qp[:, NHALF * nh:NHALF * (nh + 1)],
                            wqk_all[:, 768 * ci + 128 * t:768 * ci + 128 * (t + 1)],
                            xs(ci)[:, NHALF * nh:NHALF * (nh + 1)],
                            start=(ci == 0), stop=(ci == NCH - 1))
                for sub in range(2):
                    if t < 3:
                        head = 2 * t + sub
                        nc.scalar.copy(qaug[0:64, S * head:S * (head + 1)],
                                       qp[64 * sub:64 * sub + 64, :])
                    else:
                        head = 2 * (t - 3) + sub
                        nc.vector.tensor_copy(
                            kaug[0:64, S * head:S * (head + 1)],
                            qp[64 * sub:64 * sub + 64, :])

            for t in range(3):
                qk_octile(t)

            # ---- rel-pos bias directly into qaug rows 64:128 ----
            # h-axis: qaug[64+kh', S*i + 32*h0 + w] = sum_c rhT[c, h0+kh'] qT_i[c,(h0,w)]
            # w-axis: qaug[96+kw', S*i + 32*h + w0] = sum_c rwT[c, w0+kw'] qT_i[c,(h,w0)]
            q3h = qaug[0:64, :].rearrange("p (i x) -> p i x", i=HPC)
            q3w = qaug[0:64, :].rearrange("p (i h w) -> p i w h", i=HPC, h=H)
            bh3 = qaug[64:96, :].rearrange("p (i x) -> p i x", i=HPC)
            # w-axis results staged w-major (contiguous copies), then one
            # strided permute copy per head into qaug rows 96:128
            bw_stage = persist.tile([32, HPC * S], BF16, tag="bwst", name="bw_stage")
            bw3 = bw_stage.rearrange("p (i w h) -> p i w h", i=HPC, w=W)

            def bias_chunk(axis, trange):
                for t in trange:
                    bps = (pa if t % 2 else pb).tile(
                        [32, 2 * HPC * 32], F32,
                        tag="big" if t % 2 else "av", name="bps")
                    for j in range(2):
                        d0 = 2 * t + j
                        if axis == 0:
                            nc.tensor.matmul(
                                bps[:, 192 * j:192 * (j + 1)],
                                rhT_sb[:, d0:d0 + 32],
                                q3h[:, :, 32 * d0:32 * (d0 + 1)],
                                start=True, stop=True)
                        else:
                            nc.tensor.matmul(
                                bps[:, 192 * j:192 * (j + 1)],
                                rwT_sb[:, d0:d0 + 32],
                                q3w[:, :, d0, :],
                                start=True, stop=True)
                    for j in range(2):
                        d0 = 2 * t + j
                        src = bps[:, 192 * j:192 * (j + 1)].rearrange(
                            "p (i x) -> p i x", i=HPC)
                        dst = bh3[:, :, 32 * d0:32 * (d0 + 1)] if axis == 0 \
                            else bw3[:, :, d0, :]
                        (nc.scalar.copy if j == 0 else nc.vector.tensor_copy)(
                            dst, src)

            # interleave the k octiles with the bias chunks to keep PE busy
            qk_octile(3)
            bias_chunk(0, range(0, 8))
            qk_octile(4)
            bias_chunk(0, range(8, 16))
            qk_octile(5)
            bias_chunk(1, range(0, 16))
            # permute the staged w-axis bias (w-major -> h-major) per head
            for i in range(HPC):
                src = bw_stage[:, S * i:S * (i + 1)].rearrange(
                    "p (w h) -> p h w", w=W)
                (nc.scalar.copy if i % 2 else nc.vector.tensor_copy)(
                    qaug[96:128, S * i:S * (i + 1)], src)
            # bridge the copy-drain seam with PE work so the clock gate
            # stays open into the attention phase
            for _ in range(6):
                wp_ = pa.tile([128, NHALF], F32, tag="big", name="wp_")
                nc.tensor.matmul(wp_[:], kaug[:, 0:128], kaug[:, 0:NHALF],
                                 start=True, stop=True)

            # ---- attention: heads in interleaved pairs ----
            # av matmuls lag the score matmuls by TWO k-blocks so they never
            # wait on an exp, and each pair's two trailing av rounds overlap
            # the next pair's first two score rounds: the PE stream across
            # the whole attention phase has no dependency bubbles (keeps the
            # HAM clock gate at 2.4 GHz).
            out_headsT = [persist.tile([128, S], BF16, tag=f"ohT{c}",
                                       name=f"ohT{c}")
                          for c in range(HPC * HD // 128)]
            npairs = HPC // 2
            e_tiles = {}
            av_tiles = {}

            def sc_unit(pair, r):
                for i in (2 * pair, 2 * pair + 1):
                    sc = pa.tile([128, S], F32, tag="big", name="sc")
                    for nh in range(S // NHALF):
                        sl = slice(NHALF * nh, NHALF * (nh + 1))
                        nc.tensor.matmul(
                            sc[:, sl],
                            kaug[:, S * i + 128 * r:S * i + 128 * (r + 1)],
                            qaug[:, S * i + NHALF * nh:S * i + NHALF * (nh + 1)],
                            start=True, stop=True)
                    if i == 2 * pair:
                        e = et_pool.tile([128, S], BF16, tag="eta", name="eta",
                                         bufs=3)
                        nc.scalar.activation(
                            e[:], sc[:], mybir.ActivationFunctionType.Exp)
                        e_tiles[(i, r)] = e[:]
                    else:
                        ei = et_pool.tile([128, S], I16, tag="etb", name="etb",
                                          bufs=3)
                        nc.vector.tensor_scalar(
                            ei[:], sc[:], EXP_A, EXP_B,
                            op0=mybir.AluOpType.mult,
                            op1=mybir.AluOpType.add)
                        e_tiles[(i, r)] = ei[:].bitcast(BF16)

            def av_unit(pair, r):
                for i in (2 * pair, 2 * pair + 1):
                    if r == 0:
                        av_tiles[i] = pb.tile([HD + 1, S], F32, tag="av",
                                              name=f"av{i}")
                    e = e_tiles.pop((i, r))
                    for nh in range(S // NHALF):
                        sl = slice(NHALF * nh, NHALF * (nh + 1))
                        nc.tensor.matmul(
                            av_tiles[i][:, sl],
                            v_sb[r][:, (HD + 1) * i:(HD + 1) * (i + 1)],
                            e[:, sl],
                            start=(r == 0), stop=(r == NKB - 1))

            def norm(pair):
                # normalize: free PSUM fast via an SBUF copy, reshape the
                # denominator row across partitions by DMA, reciprocal,
                # broadcast (gpsimd; PE K=1 matmul for the last pair, when
                # the PE is otherwise idle), multiply.
                last = pair == npairs - 1
                for i in (2 * pair, 2 * pair + 1):
                    av = av_tiles.pop(i)
                    av_sb = small.tile([HD + 1, S], F32, tag="av_sb", name="av_sb")
                    nc.scalar.copy(av_sb[:], av[:])
                    den_t = small.tile([128, NQB], F32, tag="den_t", name="den_t")
                    nc.sync.dma_start(den_t[:], av_sb[HD:HD + 1, :])
                    rec_t = small.tile([128, NQB], F32, tag="rec_t", name="rec_t")
                    nc.vector.reciprocal(rec_t[:], den_t[:])
                    rec_b = small.tile([128, NQB], BF16, tag="rec_b", name="rec_b")
                    nc.vector.tensor_copy(rec_b[:], rec_t[:])
                    den_row = small.tile([1, S], BF16, tag="den_row", name="den_row")
                    nc.sync.dma_start(den_row[:], rec_b[:])
                    chunk, row = i // 2, (i % 2) * 64
                    if last:
                        rbp = pa.tile([64, S], F32, tag="big", name="rbp")
                        for nh in range(S // NHALF):
                            sl = slice(NHALF * nh, NHALF * (nh + 1))
                            nc.tensor.matmul(rbp[:, sl], ones_sb[:],
                                             den_row[:, sl], start=True, stop=True)
                        nc.vector.tensor_tensor(
                            out_headsT[chunk][row:row + 64, :], av_sb[0:HD, :],
                            rbp[:], op=mybir.AluOpType.mult)
                    else:
                        rb = small.tile([64, S], BF16, tag="rbcast", name="rbcast")
                        nc.gpsimd.partition_broadcast(rb[:], den_row[:])
                        nc.vector.tensor_tensor(
                            out_headsT[chunk][row:row + 64, :], av_sb[0:HD, :],
                            rb[:], op=mybir.AluOpType.mult)

            for s in range(8 * npairs + 2):
                for pair in range(npairs):
                    if 0 <= s - 8 * pair <= 7:
                        sc_unit(pair, s - 8 * pair)
                for pair in range(npairs):
                    r_av = s - 8 * pair - 2
                    if 0 <= r_av <= 7:
                        av_unit(pair, r_av)
                        if r_av == 7:
                            norm(pair)

            # keep the PE clock warm while the last pair's normalization
            # chain drains (these overlap it, they don't delay anything)
            scratch = persist.tile([128, NHALF], BF16, tag="scratch", name="scratch")
            for _ in range(10):
                wp_ = pa.tile([128, NHALF], F32, tag="big", name="wp_")
                nc.tensor.matmul(wp_[:], kaug[:, 0:128], kaug[:, 0:NHALF],
                                 start=True, stop=True)

            # ---- output projection (partial) ----
            for qb in range(NQB):
                pp = pa.tile([128, S], F32, tag="big", name="pp")
                for ci in range(HPC * HD // 128):
                    nc.tensor.matmul(
                        pp[:, 0:NHALF],
                        out_headsT[ci][:, 128 * qb:128 * (qb + 1)],
                        wproj_all[:, C * ci:C * ci + NHALF],
                        start=(ci == 0), stop=(ci == 2))
                    nc.tensor.matmul(
                        pp[:, NHALF:C],
                        out_headsT[ci][:, 128 * qb:128 * (qb + 1)],
                        wproj_all[:, C * ci + NHALF:C * (ci + 1)],
                        start=(ci == 0), stop=(ci == 2))
                pp_sb = small.tile([128, C], BF16, tag="pp_sb", name="pp_sb")
                (nc.scalar.copy if qb % 2 else nc.vector.tensor_copy)(
                    pp_sb[:], pp[:, 0:C])
                (nc.sync if qb % 2 else nc.scalar).dma_start(
                    out[128 * qb:128 * (qb + 1), :], pp_sb[:])

    nc.compile()
    return nc


def shard_inputs(x, Wqkv, Wproj, rel_pos_h, rel_pos_w):
    """Build the 8 per-core input maps."""
    import ml_dtypes
    bf16 = ml_dtypes.bfloat16
    scale = HD ** (-0.5)
    x = np.asarray(x, dtype=np.float32)
    Wqkv = np.asarray(Wqkv, dtype=np.float32)
    Wproj = np.asarray(Wproj, dtype=np.float32)
    rhT = np.ascontiguousarray(np.asarray(rel_pos_h, np.float32).T).astype(bf16)
    rwT = np.ascontiguousarray(np.asarray(rel_pos_w, np.float32).T).astype(bf16)
    oh = np.zeros((65, S), np.float32)
    for khp in range(H):
        oh[khp, (31 - khp) * W:(31 - khp) * W + W] = 1.0
    for kwp in range(W):
        oh[32 + kwp, 31 - kwp::W] = 1.0
    oh[64, :] = 1.0
    oh = oh.astype(bf16)
    oh6 = np.ascontiguousarray(np.tile(oh[0:64, :], (1, HPC)))

    def chunk(a):
        # (n*128, m) -> (128, n*m) with 128-row chunks side by side
        n = a.shape[0] // 128
        return np.ascontiguousarray(
            a.reshape(n, 128, a.shape[1]).transpose(1, 0, 2).reshape(
                128, n * a.shape[1])).astype(bf16)

    in_maps = []
    for core in range(NCORES):
        b = core // 2
        h0 = (core % 2) * HPC
        xb = x[b].reshape(S, C)
        xT = chunk(np.ascontiguousarray(xb.T))
        wq = Wqkv[:, h0 * HD:(h0 + HPC) * HD]
        wk = Wqkv[:, C + h0 * HD:C + (h0 + HPC) * HD] * scale
        wqk = chunk(np.concatenate([wq, wk], axis=1))
        wv = chunk(Wqkv[:, 2 * C + h0 * HD:2 * C + (h0 + HPC) * HD])
        wp = chunk(Wproj[h0 * HD:(h0 + HPC) * HD, :])
        in_maps.append({"xT": xT, "wqk": wqk, "wv": wv, "wproj": wp,
                        "rhT": rhT, "rwT": rwT, "onehot": oh, "oh6": oh6})
    return in_maps


_NC_CACHE = {}


def kernel(x, Wqkv, Wproj, bproj, rel_pos_h, rel_pos_w):
    if "nc" not in _NC_CACHE:
        _NC_CACHE["nc"] = build_program()
    nc = _NC_CACHE["nc"]
    in_maps = shard_inputs(x, Wqkv, Wproj, rel_pos_h, rel_pos_w)
    res = run_bass_kernel_spmd(nc, in_maps, list(range(NCORES)))
    bproj = np.asarray(bproj, dtype=np.float32)
    out = np.empty((B, H, W, C), dtype=np.float32)
    for b in range(B):
        acc = (res.results[2 * b]["out"].astype(np.float32)
               + res.results[2 * b + 1]["out"].astype(np.float32) + bproj)
        out[b] = acc.reshape(H, W, C)
    return out


# revision 32
# speedup vs baseline: 1.0195x; 1.0195x over previous
"""Trainium2 Bass kernel for ViTDet-style attention with decomposed
relative-position bias.

Problem shapes (hardcoded):
  x: (4, 32, 32, 768) f32, Wqkv: (768, 2304), Wproj: (768, 768),
  bproj: (768,), rel_pos_h/w: (63, 64).
  12 heads, head_dim 64, S = 32*32 = 1024.

Sharding: 48 (batch, head) pairs -> 6 heads per core, all of one batch per
core-pair. Each core computes its heads' attention and a partial output
projection (its heads' channel rows of Wproj); the host sums the two
partials (bf16) per batch and adds bproj.

Device algorithm per core (bf16 matmuls, fp32 PSUM accumulation):
  - inputs land in a handful of large strided DMAs (the sync engine's
    per-DMA issue cost, ~0.7us, was serializing the load phase)
  - qkT = Wqk^T @ x^T  (x^T supplied pre-transposed by host; k pre-scaled)
  - v   = x @ Wv       (natural layout, with an appended ones column)
  - rel-pos bias computed directly per h/w block with the rel table slice
    as stationary and a 3D strided AP moving over all 6 heads at once;
    the w-axis is staged w-major (contiguous copies) and permuted once
    per head.
  - scoresT (k x q) = kaugT^T @ qaugT in ONE K=128 matmul per tile:
    aug rows 0-63 = kT / qT, 64-95 = one-hot(h) / BhT, 96-127 = one-hot(w)/BwT
    (one-hot rows DMA'd straight from DRAM into kaug).
  - eT = exp(scoresT): head A of each pair on ScalarE (table exp), head B
    on VectorE via the Schraudolph bit trick (i16 = round(a*x+b) viewed as
    bf16) so the two exps of a round run on different engines.
  - avT (65 x q) accumulates v_aug^T-matmul over k blocks; row 64 = softmax
    denominator via the ones column.  Head pairs are interleaved with av
    matmuls one k-block behind the score matmuls.
  - normalize on-chip: copy av to SBUF (frees PSUM fast), SBUF->SBUF DMA
    reshapes the denominator row to (128,8), DVE reciprocal + cast to bf16,
    DMA back to a row, gpsimd partition-broadcast (last pair: PE K=1
    broadcast, since the PE is idle there), DVE multiply.
  - partial = out_heads @ Wproj_shard, emitted bf16 to DRAM.
"""

import numpy as np

import concourse.bass as bass
import concourse.bacc as bacc
import concourse.mybir as mybir
import concourse.tile as tile
from concourse.bass_utils import run_bass_kernel_spmd

F32 = mybir.dt.float32
BF16 = mybir.dt.bfloat16
I16 = mybir.dt.int16

NH = 12          # total heads
C = 768
HD = 64
H = W = 32
S = H * W        # 1024
B = 4
NCORES = 8
HPC = NH * B // NCORES   # heads per core = 6
NCH = 6                  # C // 128 input-channel chunks
NKB = S // 128           # 8 k blocks
NQB = S // 128           # 8 q blocks
NHALF = 512              # matmul moving-dim half

# Schraudolph exp constants for bf16 output: bits = round(EXP_A*x + EXP_B)
EXP_A = 184.66502304     # 2^7 / ln 2
EXP_B = 16247.75


def build_program():
    nc = bacc.Bacc("TRN2", target_bir_lowering=False, debug=False)

    # inputs pre-chunked on the host so every DMA is a contiguous DRAM read
    xT = nc.declare_dram_parameter("xT", [128, NCH * S], BF16, isOutput=False)
    wqk = nc.declare_dram_parameter("wqk", [128, NCH * 2 * HPC * HD], BF16,
                                    isOutput=False)
    wv = nc.declare_dram_parameter("wv", [128, NCH * HPC * HD], BF16,
                                   isOutput=False)
    wproj = nc.declare_dram_parameter("wproj", [128, 3 * C], BF16, isOutput=False)
    rhT = nc.declare_dram_parameter("rhT", [HD, 2 * H - 1], BF16, isOutput=False)
    rwT = nc.declare_dram_parameter("rwT", [HD, 2 * W - 1], BF16, isOutput=False)
    onehot = nc.declare_dram_parameter("onehot", [65, S], BF16, isOutput=False)
    oh6 = nc.declare_dram_parameter("oh6", [64, HPC * S], BF16, isOutput=False)
    out = nc.declare_dram_parameter("out", [S, C], BF16, isOutput=True)

    with tile.TileContext(nc) as tc:
        with (
            tc.tile_pool(name="persist", bufs=1) as persist,
            tc.tile_pool(name="pa", bufs=2, space="PSUM") as pa,
            tc.tile_pool(name="pb", bufs=2, space="PSUM") as pb,
            tc.tile_pool(name="et", bufs=4) as et_pool,
            tc.tile_pool(name="small", bufs=2) as small,
        ):
            # all-heads augmented k/q tiles (128, 6*S)
            kaug = persist.tile([128, HPC * S], BF16, tag="kaug", name="kaug")
            qaug = persist.tile([128, HPC * S], BF16, tag="qaug", name="qaug")
            ones_sb = persist.tile([1, 64], BF16, tag="ones", name="ones_sb")
            nc.vector.memset(ones_sb[:], 1.0)

            # ---- persistent SBUF loads: few big strided DMAs ----
            xT_all = persist.tile([128, NCH * S], BF16, tag="xT", name="xT_all")
            wv_all = persist.tile([128, NCH * HPC * HD], BF16, tag="wv", name="wv_all")
            wqk_all = persist.tile([128, NCH * 2 * HPC * HD], BF16, tag="wqk",
                                   name="wqk_all")
            wproj_all = persist.tile([128, 3 * C], BF16, tag="wproj",
                                     name="wproj_all")
            # spread the loads across per-engine DMA queues so the
            # transfers run in parallel instead of serializing on one queue
            nc.sync.dma_start(xT_all[:, 0:3 * S], xT[:, 0:3 * S])
            nc.sync.dma_start(xT_all[:, 3 * S:6 * S], xT[:, 3 * S:6 * S])
            nc.scalar.dma_start(wqk_all[:, 0:3 * 768], wqk[:, 0:3 * 768])
            nc.scalar.dma_start(wv_all[:], wv[:, :])
            nc.gpsimd.dma_start(wqk_all[:, 3 * 768:], wqk[:, 3 * 768:])
            oh = persist.tile([65, S], BF16, tag="onehot", name="onehot")
            nc.sync.dma_start(oh[:], onehot[:, :])
            rhT_sb = persist.tile([HD, 2 * H - 1], BF16, tag="rhT", name="rhT_sb")
            nc.sync.dma_start(rhT_sb[:], rhT[:, :])
            rwT_sb = persist.tile([HD, 2 * W - 1], BF16, tag="rwT", name="rwT_sb")
            nc.sync.dma_start(rwT_sb[:], rwT[:, :])
            # off the critical path: issue from the (idle) gpsimd queue
            nc.gpsimd.dma_start(wproj_all[:], wproj[:, :])
            nc.gpsimd.dma_start(kaug[64:128, :], oh6[:, :])

            def xs(ci):
                return xT_all[:, S * ci:S * (ci + 1)]

            # ---- v projection (natural) + ones column ----
            # v_sb[sb]: (128, 6*65) cols [65i..65i+64) = head i v, col 65i+64 = 1
            # emitted interleaved into the qk/bias phase (v is first needed
            # only when the attention av units start)
            v_sb = [persist.tile([128, HPC * (HD + 1)], BF16, tag=f"v{sb}", name=f"v{sb}")
                    for sb in range(NKB)]

            def v_unit(sb):
                vp = pa.tile([128, HPC * HD + HPC], F32, tag="big", name="vp")
                for ci in range(NCH):
                    nc.tensor.matmul(
                        vp[:, 0:HPC * HD],
                        xs(ci)[:, 128 * sb:128 * (sb + 1)],
                        wv_all[:, HPC * HD * ci:HPC * HD * (ci + 1)],
                        start=(ci == 0), stop=(ci == NCH - 1))
                nc.tensor.matmul(vp[:, HPC * HD:HPC * HD + HPC],
                                 oh[64:65, 128 * sb:128 * (sb + 1)],
                                 oh[64:65, 0:HPC], start=True, stop=True)
                vdst = v_sb[sb].rearrange("p (i c) -> p i c", c=HD + 1)
                nc.vector.tensor_copy(
                    vdst[:, :, 0:HD],
                    vp[:, 0:HPC * HD].rearrange("p (i c) -> p i c", c=HD))
                nc.vector.tensor_copy(
                    vdst[:, :, HD:HD + 1],
                    vp[:, HPC * HD:HPC * HD + HPC].rearrange("p (i c) -> p i c", c=1))

            # ---- qk projection (transposed layout) ----
            # octile t covers oc rows [128t, 128t+128): t<3 -> q, t>=3 -> k
            def qk_octile(t):
                qp = pa.tile([128, S], F32, tag="big", name="qp")
                for ci in range(NCH):
                    for nh in range(S // NHALF):
                        nc.tensor.matmul(
                            qp[:, NHALF * nh:NHALF * (nh + 1)],
                            wqk_all[:, 768 * ci + 128 * t:768 * ci + 128 * (t + 1)],
                            xs(ci)[:, NHALF * nh:NHALF * (nh + 1)],
                            start=(ci == 0), stop=(ci == NCH - 1))
                for sub in range(2):
                    if t < 3:
                        head = 2 * t + sub
                        nc.scalar.copy(qaug[0:64, S * head:S * (head + 1)],
                                       qp[64 * sub:64 * sub + 64, :])
                    else:
                        head = 2 * (t - 3) + sub
                        nc.vector.tensor_copy(
                            kaug[0:64, S * head:S * (head + 1)],
                            qp[64 * sub:64 * sub + 64, :])

            for t in range(3):
                qk_octile(t)

            # ---- rel-pos bias directly into qaug rows 64:128 ----
            # h-axis: qaug[64+kh', S*i + 32*h0 + w] = sum_c rhT[c, h0+kh'] qT_i[c,(h0,w)]
            # w-axis: qaug[96+kw', S*i + 32*h + w0] = sum_c rwT[c, w0+kw'] qT_i[c,(h,w0)]
            q3h = qaug[0:64, :].rearrange("p (i x) -> p i x", i=HPC)
            q3w = qaug[0:64, :].rearrange("p (i h w) -> p i w h", i=HPC, h=H)
            bh3 = qaug[64:96, :].rearrange("p (i x) -> p i x", i=HPC)
            # w-axis results staged w-major (contiguous copies), then one
            # strided permute copy per head into qaug rows 96:128
            bw_stage = persist.tile([32, HPC * S], BF16, tag="bwst", name="bw_stage")
            bw3 = bw_stage.rearrange("p (i w h) -> p i w h", i=HPC, w=W)

            def bias_chunk(axis, trange):
                for t in trange:
                    bps = (pa if t % 2 else pb).tile(
                        [32, 2 * HPC * 32], F32,
                        tag="big" if t % 2 else "av", name="bps")
                    for j in range(2):
                        d0 = 2 * t + j
                        if axis == 0:
                            nc.tensor.matmul(
                                bps[:, 192 * j:192 * (j + 1)],
                                rhT_sb[:, d0:d0 + 32],
                                q3h[:, :, 32 * d0:32 * (d0 + 1)],
                                start=True, stop=True)
                        else:
                            nc.tensor.matmul(
                                bps[:, 192 * j:192 * (j + 1)],
                                rwT_sb[:, d0:d0 + 32],
                                q3w[:, :, d0, :],
                                start=True, stop=True)
                    for j in range(2):
                        d0 = 2 * t + j
                        src = bps[:, 192 * j:192 * (j + 1)].rearrange(
                            "p (i x) -> p i x", i=HPC)
                        dst = bh3[:, :, 32 * d0:32 * (d0 + 1)] if axis == 0 \
                            else bw3[:, :, d0, :]
                        (nc.scalar.copy if j == 0 else nc.vector.tensor_copy)(
                            dst, src)

            # interleave the k octiles, v units, and bias chunks: the v
            # matmuls fill the PE while the bias copies drain
            qk_octile(3)
            v_unit(0)
            v_unit(1)
            bias_chunk(0, range(0, 8))
            qk_octile(4)
            v_unit(2)
            v_unit(3)
            bias_chunk(0, range(8, 16))
            qk_octile(5)
            v_unit(4)
            v_unit(5)
            bias_chunk(1, range(0, 8))
            v_unit(6)
            v_unit(7)
            bias_chunk(1, range(8, 16))
            # permute the staged w-axis bias (w-major -> h-major) per head
            for i in range(HPC):
                src = bw_stage[:, S * i:S * (i + 1)].rearrange(
                    "p (w h) -> p h w", w=W)
                (nc.scalar.copy if i % 2 else nc.vector.tensor_copy)(
                    qaug[96:128, S * i:S * (i + 1)], src)
            # bridge the copy-drain seam with PE work so the clock gate
            # stays open into the attention phase
            for _ in range(6):
                wp_ = pa.tile([128, NHALF], F32, tag="big", name="wp_")
                nc.tensor.matmul(wp_[:], kaug[:, 0:128], kaug[:, 0:NHALF],
                                 start=True, stop=True)

            # ---- attention: heads in interleaved pairs ----
            # av matmuls lag the score matmuls by TWO k-blocks so they never
            # wait on an exp, and each pair's two trailing av rounds overlap
            # the next pair's first two score rounds: the PE stream across
            # the whole attention phase has no dependency bubbles (keeps the
            # HAM clock gate at 2.4 GHz).
            out_headsT = [persist.tile([128, S], BF16, tag=f"ohT{c}",
                                       name=f"ohT{c}")
                          for c in range(HPC * HD // 128)]
            npairs = HPC // 2
            e_tiles = {}
            av_tiles = {}

            def sc_unit(pair, r):
                for i in (2 * pair, 2 * pair + 1):
                    sc = pa.tile([128, S], F32, tag="big", name="sc")
                    for nh in range(S // NHALF):
                        sl = slice(NHALF * nh, NHALF * (nh + 1))
                        nc.tensor.matmul(
                            sc[:, sl],
                            kaug[:, S * i + 128 * r:S * i + 128 * (r + 1)],
                            qaug[:, S * i + NHALF * nh:S * i + NHALF * (nh + 1)],
                            start=True, stop=True)
                    if i == 2 * pair:
                        e = et_pool.tile([128, S], BF16, tag="eta", name="eta",
                                         bufs=3)
                        nc.scalar.activation(
                            e[:], sc[:], mybir.ActivationFunctionType.Exp)
                        e_tiles[(i, r)] = e[:]
                    else:
                        ei = et_pool.tile([128, S], I16, tag="etb", name="etb",
                                          bufs=3)
                        nc.vector.tensor_scalar(
                            ei[:], sc[:], EXP_A, EXP_B,
                            op0=mybir.AluOpType.mult,
                            op1=mybir.AluOpType.add)
                        e_tiles[(i, r)] = ei[:].bitcast(BF16)

            def av_unit(pair, r):
                for i in (2 * pair, 2 * pair + 1):
                    if r == 0:
                        av_tiles[i] = pb.tile([HD + 1, S], F32, tag="av",
                                              name=f"av{i}")
                    e = e_tiles.pop((i, r))
                    for nh in range(S // NHALF):
                        sl = slice(NHALF * nh, NHALF * (nh + 1))
                        nc.tensor.matmul(
                            av_tiles[i][:, sl],
                            v_sb[r][:, (HD + 1) * i:(HD + 1) * (i + 1)],
                            e[:, sl],
                            start=(r == 0), stop=(r == NKB - 1))

            def norm(pair):
                # normalize: free PSUM fast via an SBUF copy, reshape the
                # denominator row across partitions by DMA, reciprocal,
                # broadcast (gpsimd; PE K=1 matmul for the last pair, when
                # the PE is otherwise idle), multiply.
                last = pair == npairs - 1
                for i in (2 * pair, 2 * pair + 1):
                    av = av_tiles.pop(i)
                    av_sb = small.tile([HD + 1, S], F32, tag="av_sb", name="av_sb")
                    nc.scalar.copy(av_sb[:], av[:])
                    den_t = small.tile([128, NQB], F32, tag="den_t", name="den_t")
                    nc.sync.dma_start(den_t[:], av_sb[HD:HD + 1, :])
                    rec_t = small.tile([128, NQB], F32, tag="rec_t", name="rec_t")
                    nc.vector.reciprocal(rec_t[:], den_t[:])
                    rec_b = small.tile([128, NQB], BF16, tag="rec_b", name="rec_b")
                    nc.vector.tensor_copy(rec_b[:], rec_t[:])
                    den_row = small.tile([1, S], BF16, tag="den_row", name="den_row")
                    nc.sync.dma_start(den_row[:], rec_b[:])
                    chunk, row = i // 2, (i % 2) * 64
                    if last:
                        rbp = pa.tile([64, S], F32, tag="big", name="rbp")
                        for nh in range(S // NHALF):
                            sl = slice(NHALF * nh, NHALF * (nh + 1))
                            nc.tensor.matmul(rbp[:, sl], ones_sb[:],
                                             den_row[:, sl], start=True, stop=True)
                        nc.vector.tensor_tensor(
                            out_headsT[chunk][row:row + 64, :], av_sb[0:HD, :],
                            rbp[:], op=mybir.AluOpType.mult)
                    else:
                        rb = small.tile([64, S], BF16, tag="rbcast", name="rbcast")
                        nc.gpsimd.partition_broadcast(rb[:], den_row[:])
                        nc.vector.tensor_tensor(
                            out_headsT[chunk][row:row + 64, :], av_sb[0:HD, :],
                            rb[:], op=mybir.AluOpType.mult)

            for s in range(8 * npairs + 2):
                for pair in range(npairs):
                    if 0 <= s - 8 * pair <= 7:
                        sc_unit(pair, s - 8 * pair)
                for pair in range(npairs):
                    r_av = s - 8 * pair - 2
                    if 0 <= r_av <= 7:
                        av_unit(pair, r_av)
                        if r_av == 7:
                            norm(pair)

            # keep the PE clock warm while the last pair's normalization
            # chain drains (these overlap it, they don't delay anything)
            scratch = persist.tile([128, NHALF], BF16, tag="scratch", name="scratch")
            for _ in range(10):
                wp_ = pa.tile([128, NHALF], F32, tag="big", name="wp_")
                nc.tensor.matmul(wp_[:], kaug[:, 0:128], kaug[:, 0:NHALF],
                                 start=True, stop=True)

            # ---- output projection (partial) ----
            for qb in range(NQB):
                pp = pa.tile([128, S], F32, tag="big", name="pp")
                for ci in range(HPC * HD // 128):
                    nc.tensor.matmul(
                        pp[:, 0:NHALF],
                        out_headsT[ci][:, 128 * qb:128 * (qb + 1)],
                        wproj_all[:, C * ci:C * ci + NHALF],
                        start=(ci == 0), stop=(ci == 2))
                    nc.tensor.matmul(
                        pp[:, NHALF:C],
                        out_headsT[ci][:, 128 * qb:128 * (qb + 1)],
                        wproj_all[:, C * ci + NHALF:C * (ci + 1)],
                        start=(ci == 0), stop=(ci == 2))
                pp_sb = small.tile([128, C], BF16, tag="pp_sb", name="pp_sb")
                (nc.scalar.copy if qb % 2 else nc.vector.tensor_copy)(
                    pp_sb[:], pp[:, 0:C])
                (nc.sync if qb % 2 else nc.scalar).dma_start(
                    out[128 * qb:128 * (qb + 1), :], pp_sb[:])

    nc.compile()
    return nc


def shard_inputs(x, Wqkv, Wproj, rel_pos_h, rel_pos_w):
    """Build the 8 per-core input maps."""
    import ml_dtypes
    bf16 = ml_dtypes.bfloat16
    scale = HD ** (-0.5)
    x = np.asarray(x, dtype=np.float32)
    Wqkv = np.asarray(Wqkv, dtype=np.float32)
    Wproj = np.asarray(Wproj, dtype=np.float32)
    rhT = np.ascontiguousarray(np.asarray(rel_pos_h, np.float32).T).astype(bf16)
    rwT = np.ascontiguousarray(np.asarray(rel_pos_w, np.float32).T).astype(bf16)
    oh = np.zeros((65, S), np.float32)
    for khp in range(H):
        oh[khp, (31 - khp) * W:(31 - khp) * W + W] = 1.0
    for kwp in range(W):
        oh[32 + kwp, 31 - kwp::W] = 1.0
    oh[64, :] = 1.0
    oh = oh.astype(bf16)
    oh6 = np.ascontiguousarray(np.tile(oh[0:64, :], (1, HPC)))

    def chunk(a):
        # (n*128, m) -> (128, n*m) with 128-row chunks side by side
        n = a.shape[0] // 128
        return np.ascontiguousarray(
            a.reshape(n, 128, a.shape[1]).transpose(1, 0, 2).reshape(
                128, n * a.shape[1])).astype(bf16)

    in_maps = []
    for core in range(NCORES):
        b = core // 2
        h0 = (core % 2) * HPC
        xb = x[b].reshape(S, C)
        xT = chunk(np.ascontiguousarray(xb.T))
        wq = Wqkv[:, h0 * HD:(h0 + HPC) * HD]
        wk = Wqkv[:, C + h0 * HD:C + (h0 + HPC) * HD] * scale
        wqk = chunk(np.concatenate([wq, wk], axis=1))
        wv = chunk(Wqkv[:, 2 * C + h0 * HD:2 * C + (h0 + HPC) * HD])
        wp = chunk(Wproj[h0 * HD:(h0 + HPC) * HD, :])
        in_maps.append({"xT": xT, "wqk": wqk, "wv": wv, "wproj": wp,
                        "rhT": rhT, "rwT": rwT, "onehot": oh, "oh6": oh6})
    return in_maps


_NC_CACHE = {}


def kernel(x, Wqkv, Wproj, bproj, rel_pos_h, rel_pos_w):
    if "nc" not in _NC_CACHE:
        _NC_CACHE["nc"] = build_program()
    nc = _NC_CACHE["nc"]
    in_maps = shard_inputs(x, Wqkv, Wproj, rel_pos_h, rel_pos_w)
    res = run_bass_kernel_spmd(nc, in_maps, list(range(NCORES)))
    bproj = np.asarray(bproj, dtype=np.float32)
    out = np.empty((B, H, W, C), dtype=np.float32)
    for b in range(B):
        acc = (res.results[2 * b]["out"].astype(np.float32)
               + res.results[2 * b + 1]["out"].astype(np.float32) + bproj)
        out[b] = acc.reshape(H, W, C)
    return out


# revision 33
# speedup vs baseline: 1.1290x; 1.1074x over previous
"""Trainium2 Bass kernel for ViTDet-style attention with decomposed
relative-position bias.

Problem shapes (hardcoded):
  x: (4, 32, 32, 768) f32, Wqkv: (768, 2304), Wproj: (768, 768),
  bproj: (768,), rel_pos_h/w: (63, 64).
  12 heads, head_dim 64, S = 32*32 = 1024.

Sharding: 48 (batch, head) pairs -> 6 heads per core, all of one batch per
core-pair. Each core computes its heads' attention and a partial output
projection (its heads' channel rows of Wproj); the host sums the two
partials (bf16) per batch and adds bproj.

Device algorithm per core (bf16 matmuls, fp32 PSUM accumulation):
  - inputs land in a handful of large strided DMAs (the sync engine's
    per-DMA issue cost, ~0.7us, was serializing the load phase)
  - qkT = Wqk^T @ x^T  (x^T supplied pre-transposed by host; k pre-scaled)
  - v   = x @ Wv       (natural layout, with an appended ones column)
  - rel-pos bias computed directly per h/w block with the rel table slice
    as stationary and a 3D strided AP moving over all 6 heads at once;
    the w-axis is staged w-major (contiguous copies) and permuted once
    per head.
  - scoresT (k x q) = kaugT^T @ qaugT in ONE K=128 matmul per tile:
    aug rows 0-63 = kT / qT, 64-95 = one-hot(h) / BhT, 96-127 = one-hot(w)/BwT
    (one-hot rows DMA'd straight from DRAM into kaug).
  - eT = exp(scoresT): head A of each pair on ScalarE (table exp), head B
    on VectorE via the Schraudolph bit trick (i16 = round(a*x+b) viewed as
    bf16) so the two exps of a round run on different engines.
  - avT (65 x q) accumulates v_aug^T-matmul over k blocks; row 64 = softmax
    denominator via the ones column.  Head pairs are interleaved with av
    matmuls one k-block behind the score matmuls.
  - normalize on-chip: copy av to SBUF (frees PSUM fast), SBUF->SBUF DMA
    reshapes the denominator row to (128,8), DVE reciprocal + cast to bf16,
    DMA back to a row, gpsimd partition-broadcast (last pair: PE K=1
    broadcast, since the PE is idle there), DVE multiply.
  - partial = out_heads @ Wproj_shard, emitted bf16 to DRAM.
"""

import numpy as np

import concourse.bass as bass
import concourse.bacc as bacc
import concourse.mybir as mybir
import concourse.tile as tile
from concourse.bass_utils import run_bass_kernel_spmd

F32 = mybir.dt.float32
BF16 = mybir.dt.bfloat16
I16 = mybir.dt.int16

NH = 12          # total heads
C = 768
HD = 64
H = W = 32
S = H * W        # 1024
B = 4
NCORES = 8
HPC = NH * B // NCORES   # heads per core = 6
NCH = 6                  # C // 128 input-channel chunks
NKB = S // 128           # 8 k blocks
NQB = S // 128           # 8 q blocks
NHALF = 512              # matmul moving-dim half

# Schraudolph exp constants for bf16 output: bits = round(EXP_A*x + EXP_B)
EXP_A = 184.66502304     # 2^7 / ln 2
EXP_B = 16247.75


def build_program():
    nc = bacc.Bacc("TRN2", target_bir_lowering=False, debug=False)

    # inputs pre-chunked on the host so every DMA is a contiguous DRAM read
    xT = nc.declare_dram_parameter("xT", [128, NCH * S], BF16, isOutput=False)
    wqk = nc.declare_dram_parameter("wqk", [128, NCH * 2 * HPC * HD], BF16,
                                    isOutput=False)
    wv = nc.declare_dram_parameter("wv", [128, NCH * HPC * HD], BF16,
                                   isOutput=False)
    wproj = nc.declare_dram_parameter("wproj", [128, 3 * C], BF16, isOutput=False)
    rhT = nc.declare_dram_parameter("rhT", [HD, 2 * H - 1], BF16, isOutput=False)
    rwT = nc.declare_dram_parameter("rwT", [HD, 2 * W - 1], BF16, isOutput=False)
    onehot = nc.declare_dram_parameter("onehot", [65, S], BF16, isOutput=False)
    oh6 = nc.declare_dram_parameter("oh6", [64, HPC * S], BF16, isOutput=False)
    out = nc.declare_dram_parameter("out", [S, C], BF16, isOutput=True)

    with tile.TileContext(nc) as tc:
        with (
            tc.tile_pool(name="persist", bufs=1) as persist,
            tc.tile_pool(name="pa", bufs=2, space="PSUM") as pa,
            tc.tile_pool(name="pb", bufs=2, space="PSUM") as pb,
            tc.tile_pool(name="et", bufs=4) as et_pool,
            tc.tile_pool(name="small", bufs=2) as small,
        ):
            # all-heads augmented k/q tiles (128, 6*S)
            kaug = persist.tile([128, HPC * S], BF16, tag="kaug", name="kaug")
            qaug = persist.tile([128, HPC * S], BF16, tag="qaug", name="qaug")
            ones_sb = persist.tile([1, 64], BF16, tag="ones", name="ones_sb")
            nc.vector.memset(ones_sb[:], 1.0)

            # ---- persistent SBUF loads: few big strided DMAs ----
            xT_all = persist.tile([128, NCH * S], BF16, tag="xT", name="xT_all")
            wv_all = persist.tile([128, NCH * HPC * HD], BF16, tag="wv", name="wv_all")
            wqk_all = persist.tile([128, NCH * 2 * HPC * HD], BF16, tag="wqk",
                                   name="wqk_all")
            wproj_all = persist.tile([128, 3 * C], BF16, tag="wproj",
                                     name="wproj_all")
            # spread the loads across per-engine DMA queues so the
            # transfers run in parallel instead of serializing on one queue
            nc.sync.dma_start(xT_all[:, 0:3 * S], xT[:, 0:3 * S])
            nc.sync.dma_start(xT_all[:, 3 * S:6 * S], xT[:, 3 * S:6 * S])
            nc.scalar.dma_start(wqk_all[:, 0:3 * 768], wqk[:, 0:3 * 768])
            nc.scalar.dma_start(wv_all[:], wv[:, :])
            nc.gpsimd.dma_start(wqk_all[:, 3 * 768:], wqk[:, 3 * 768:])
            oh = persist.tile([65, S], BF16, tag="onehot", name="onehot")
            nc.sync.dma_start(oh[:], onehot[:, :])
            rhT_sb = persist.tile([HD, 2 * H - 1], BF16, tag="rhT", name="rhT_sb")
            nc.sync.dma_start(rhT_sb[:], rhT[:, :])
            rwT_sb = persist.tile([HD, 2 * W - 1], BF16, tag="rwT", name="rwT_sb")
            nc.sync.dma_start(rwT_sb[:], rwT[:, :])
            # off the critical path: issue from the (idle) gpsimd queue
            nc.gpsimd.dma_start(wproj_all[:], wproj[:, :])
            nc.gpsimd.dma_start(kaug[64:128, :], oh6[:, :])

            def xs(ci):
                return xT_all[:, S * ci:S * (ci + 1)]

            # ---- v projection (natural) + ones column ----
            # v_sb[sb]: (128, 6*65) cols [65i..65i+64) = head i v, col 65i+64 = 1
            # emitted interleaved into the qk/bias phase (v is first needed
            # only when the attention av units start)
            v_sb = [persist.tile([128, HPC * (HD + 1)], BF16, tag=f"v{sb}", name=f"v{sb}")
                    for sb in range(NKB)]

            def v_unit(sb):
                vp = pa.tile([128, HPC * HD + HPC], F32, tag="big", name="vp")
                for ci in range(NCH):
                    nc.tensor.matmul(
                        vp[:, 0:HPC * HD],
                        xs(ci)[:, 128 * sb:128 * (sb + 1)],
                        wv_all[:, HPC * HD * ci:HPC * HD * (ci + 1)],
                        start=(ci == 0), stop=(ci == NCH - 1))
                nc.tensor.matmul(vp[:, HPC * HD:HPC * HD + HPC],
                                 oh[64:65, 128 * sb:128 * (sb + 1)],
                                 oh[64:65, 0:HPC], start=True, stop=True)
                vdst = v_sb[sb].rearrange("p (i c) -> p i c", c=HD + 1)
                nc.vector.tensor_copy(
                    vdst[:, :, 0:HD],
                    vp[:, 0:HPC * HD].rearrange("p (i c) -> p i c", c=HD))
                nc.vector.tensor_copy(
                    vdst[:, :, HD:HD + 1],
                    vp[:, HPC * HD:HPC * HD + HPC].rearrange("p (i c) -> p i c", c=1))

            # ---- qk projection (transposed layout) ----
            # octile t covers oc rows [128t, 128t+128): t<3 -> q, t>=3 -> k
            def qk_octile(t):
                qp = pa.tile([128, S], F32, tag="big", name="qp")
                for ci in range(NCH):
                    for nh in range(S // NHALF):
                        nc.tensor.matmul(
                            qp[:, NHALF * nh:NHALF * (nh + 1)],
                            wqk_all[:, 768 * ci + 128 * t:768 * ci + 128 * (t + 1)],
                            xs(ci)[:, NHALF * nh:NHALF * (nh + 1)],
                            start=(ci == 0), stop=(ci == NCH - 1))
                for sub in range(2):
                    if t < 3:
                        head = 2 * t + sub
                        nc.scalar.copy(qaug[0:64, S * head:S * (head + 1)],
                                       qp[64 * sub:64 * sub + 64, :])
                    else:
                        head = 2 * (t - 3) + sub
                        nc.vector.tensor_copy(
                            kaug[0:64, S * head:S * (head + 1)],
                            qp[64 * sub:64 * sub + 64, :])

            for t in range(3):
                qk_octile(t)

            # ---- rel-pos bias directly into qaug rows 64:128 ----
            # h-axis: qaug[64+kh', S*i + 32*h0 + w] = sum_c rhT[c, h0+kh'] qT_i[c,(h0,w)]
            # w-axis: qaug[96+kw', S*i + 32*h + w0] = sum_c rwT[c, w0+kw'] qT_i[c,(h,w0)]
            q3h = qaug[0:64, :].rearrange("p (i x) -> p i x", i=HPC)
            q3w = qaug[0:64, :].rearrange("p (i h w) -> p i w h", i=HPC, h=H)
            bh3 = qaug[64:96, :].rearrange("p (i x) -> p i x", i=HPC)
            # w-axis results staged w-major (contiguous copies), then one
            # strided permute copy per head into qaug rows 96:128
            bw_stage = persist.tile([32, HPC * S], BF16, tag="bwst", name="bw_stage")
            bw3 = bw_stage.rearrange("p (i w h) -> p i w h", i=HPC, w=W)

            def bias_chunk(axis, trange):
                for t in trange:
                    bps = (pa if t % 2 else pb).tile(
                        [32, 2 * HPC * 32], F32,
                        tag="big" if t % 2 else "av", name="bps")
                    for j in range(2):
                        d0 = 2 * t + j
                        if axis == 0:
                            nc.tensor.matmul(
                                bps[:, 192 * j:192 * (j + 1)],
                                rhT_sb[:, d0:d0 + 32],
                                q3h[:, :, 32 * d0:32 * (d0 + 1)],
                                start=True, stop=True)
                        else:
                            nc.tensor.matmul(
                                bps[:, 192 * j:192 * (j + 1)],
                                rwT_sb[:, d0:d0 + 32],
                                q3w[:, :, d0, :],
                                start=True, stop=True)
                    for j in range(2):
                        d0 = 2 * t + j
                        src = bps[:, 192 * j:192 * (j + 1)].rearrange(
                            "p (i x) -> p i x", i=HPC)
                        dst = bh3[:, :, 32 * d0:32 * (d0 + 1)] if axis == 0 \
                            else bw3[:, :, d0, :]
                        (nc.scalar.copy if j == 0 else nc.vector.tensor_copy)(
                            dst, src)

            # interleave the k octiles, v units, and bias chunks: the v
            # matmuls fill the PE while the bias copies drain
            qk_octile(3)
            v_unit(0)
            v_unit(1)
            bias_chunk(0, range(0, 8))
            qk_octile(4)
            v_unit(2)
            v_unit(3)
            bias_chunk(0, range(8, 16))
            qk_octile(5)
            v_unit(4)
            v_unit(5)
            bias_chunk(1, range(0, 8))
            v_unit(6)
            v_unit(7)
            bias_chunk(1, range(8, 16))
            # permute the staged w-axis bias (w-major -> h-major) per head
            for i in range(HPC):
                src = bw_stage[:, S * i:S * (i + 1)].rearrange(
                    "p (w h) -> p h w", w=W)
                (nc.scalar.copy if i % 2 else nc.vector.tensor_copy)(
                    qaug[96:128, S * i:S * (i + 1)], src)
            # bridge the copy-drain seam with PE work so the clock gate
            # stays open into the attention phase
            for _ in range(6):
                wp_ = pa.tile([128, NHALF], F32, tag="big", name="wp_")
                nc.tensor.matmul(wp_[:], kaug[:, 0:128], kaug[:, 0:NHALF],
                                 start=True, stop=True)

            # ---- attention: heads in interleaved pairs ----
            # av matmuls lag the score matmuls by TWO k-blocks so they never
            # wait on an exp, and each pair's two trailing av rounds overlap
            # the next pair's first two score rounds: the PE stream across
            # the whole attention phase has no dependency bubbles (keeps the
            # HAM clock gate at 2.4 GHz).
            out_headsT = [persist.tile([128, S], BF16, tag=f"ohT{c}",
                                       name=f"ohT{c}")
                          for c in range(HPC * HD // 128)]
            npairs = HPC // 2
            e_tiles = {}
            av_tiles = {}

            def sc_unit(pair, r):
                for i in (2 * pair, 2 * pair + 1):
                    sc = pa.tile([128, S], F32, tag="big", name="sc")
                    for nh in range(S // NHALF):
                        sl = slice(NHALF * nh, NHALF * (nh + 1))
                        nc.tensor.matmul(
                            sc[:, sl],
                            kaug[:, S * i + 128 * r:S * i + 128 * (r + 1)],
                            qaug[:, S * i + NHALF * nh:S * i + NHALF * (nh + 1)],
                            start=True, stop=True)
                    if i == 2 * pair:
                        e = et_pool.tile([128, S], BF16, tag="eta", name="eta",
                                         bufs=3)
                        nc.scalar.activation(
                            e[:], sc[:], mybir.ActivationFunctionType.Exp)
                        e_tiles[(i, r)] = e[:]
                    else:
                        ei = et_pool.tile([128, S], I16, tag="etb", name="etb",
                                          bufs=3)
                        nc.vector.tensor_scalar(
                            ei[:], sc[:], EXP_A, EXP_B,
                            op0=mybir.AluOpType.mult,
                            op1=mybir.AluOpType.add)
                        e_tiles[(i, r)] = ei[:].bitcast(BF16)

            def av_unit(pair, r):
                for i in (2 * pair, 2 * pair + 1):
                    if r == 0:
                        av_tiles[i] = pb.tile([HD + 1, S], F32, tag="av",
                                              name=f"av{i}")
                    e = e_tiles.pop((i, r))
                    for nh in range(S // NHALF):
                        sl = slice(NHALF * nh, NHALF * (nh + 1))
                        nc.tensor.matmul(
                            av_tiles[i][:, sl],
                            v_sb[r][:, (HD + 1) * i:(HD + 1) * (i + 1)],
                            e[:, sl],
                            start=(r == 0), stop=(r == NKB - 1))

            def norm(pair):
                # normalize: copy only the denominator ROW to SBUF, reshape
                # it across partitions by DMA, reciprocal, gpsimd broadcast,
                # then multiply straight out of the av PSUM tile.  The lag-2
                # schedule leaves ~4 slots before the av slot is reused, so
                # holding it until the multiply no longer stalls the PE.
                for i in (2 * pair, 2 * pair + 1):
                    av = av_tiles.pop(i)
                    den_sb = small.tile([1, S], F32, tag="den_sb", name="den_sb")
                    nc.scalar.copy(den_sb[:], av[HD:HD + 1, :])
                    den_t = small.tile([128, NQB], F32, tag="den_t", name="den_t")
                    nc.sync.dma_start(den_t[:], den_sb[:])
                    rec_t = small.tile([128, NQB], F32, tag="rec_t", name="rec_t")
                    nc.vector.reciprocal(rec_t[:], den_t[:])
                    rec_b = small.tile([128, NQB], BF16, tag="rec_b", name="rec_b")
                    nc.vector.tensor_copy(rec_b[:], rec_t[:])
                    den_row = small.tile([1, S], BF16, tag="den_row", name="den_row")
                    nc.sync.dma_start(den_row[:], rec_b[:])
                    chunk, row = i // 2, (i % 2) * 64
                    rb = small.tile([64, S], BF16, tag="rbcast", name="rbcast")
                    nc.gpsimd.partition_broadcast(rb[:], den_row[:])
                    nc.vector.tensor_tensor(
                        out_headsT[chunk][row:row + 64, :], av[0:HD, :],
                        rb[:], op=mybir.AluOpType.mult)

            for s in range(8 * npairs + 2):
                for pair in range(npairs):
                    if 0 <= s - 8 * pair <= 7:
                        sc_unit(pair, s - 8 * pair)
                for pair in range(npairs):
                    r_av = s - 8 * pair - 2
                    if 0 <= r_av <= 7:
                        av_unit(pair, r_av)
                        if r_av == 7:
                            norm(pair)

            # keep the PE clock warm while the last pair's normalization
            # chain drains (these overlap it, they don't delay anything)
            scratch = persist.tile([128, NHALF], BF16, tag="scratch", name="scratch")
            for _ in range(10):
                wp_ = pa.tile([128, NHALF], F32, tag="big", name="wp_")
                nc.tensor.matmul(wp_[:], kaug[:, 0:128], kaug[:, 0:NHALF],
                                 start=True, stop=True)

            # ---- output projection (partial) ----
            for qb in range(NQB):
                pp = pa.tile([128, S], F32, tag="big", name="pp")
                for ci in range(HPC * HD // 128):
                    nc.tensor.matmul(
                        pp[:, 0:NHALF],
                        out_headsT[ci][:, 128 * qb:128 * (qb + 1)],
                        wproj_all[:, C * ci:C * ci + NHALF],
                        start=(ci == 0), stop=(ci == 2))
                    nc.tensor.matmul(
                        pp[:, NHALF:C],
                        out_headsT[ci][:, 128 * qb:128 * (qb + 1)],
                        wproj_all[:, C * ci + NHALF:C * (ci + 1)],
                        start=(ci == 0), stop=(ci == 2))
                pp_sb = small.tile([128, C], BF16, tag="pp_sb", name="pp_sb")
                (nc.scalar.copy if qb % 2 else nc.vector.tensor_copy)(
                    pp_sb[:], pp[:, 0:C])
                (nc.sync if qb % 2 else nc.scalar).dma_start(
                    out[128 * qb:128 * (qb + 1), :], pp_sb[:])

    nc.compile()
    return nc


def shard_inputs(x, Wqkv, Wproj, rel_pos_h, rel_pos_w):
    """Build the 8 per-core input maps."""
    import ml_dtypes
    bf16 = ml_dtypes.bfloat16
    scale = HD ** (-0.5)
    x = np.asarray(x, dtype=np.float32)
    Wqkv = np.asarray(Wqkv, dtype=np.float32)
    Wproj = np.asarray(Wproj, dtype=np.float32)
    rhT = np.ascontiguousarray(np.asarray(rel_pos_h, np.float32).T).astype(bf16)
    rwT = np.ascontiguousarray(np.asarray(rel_pos_w, np.float32).T).astype(bf16)
    oh = np.zeros((65, S), np.float32)
    for khp in range(H):
        oh[khp, (31 - khp) * W:(31 - khp) * W + W] = 1.0
    for kwp in range(W):
        oh[32 + kwp, 31 - kwp::W] = 1.0
    oh[64, :] = 1.0
    oh = oh.astype(bf16)
    oh6 = np.ascontiguousarray(np.tile(oh[0:64, :], (1, HPC)))

    def chunk(a):
        # (n*128, m) -> (128, n*m) with 128-row chunks side by side
        n = a.shape[0] // 128
        return np.ascontiguousarray(
            a.reshape(n, 128, a.shape[1]).transpose(1, 0, 2).reshape(
                128, n * a.shape[1])).astype(bf16)

    in_maps = []
    for core in range(NCORES):
        b = core // 2
        h0 = (core % 2) * HPC
        xb = x[b].reshape(S, C)
        xT = chunk(np.ascontiguousarray(xb.T))
        wq = Wqkv[:, h0 * HD:(h0 + HPC) * HD]
        wk = Wqkv[:, C + h0 * HD:C + (h0 + HPC) * HD] * scale
        wqk = chunk(np.concatenate([wq, wk], axis=1))
        wv = chunk(Wqkv[:, 2 * C + h0 * HD:2 * C + (h0 + HPC) * HD])
        wp = chunk(Wproj[h0 * HD:(h0 + HPC) * HD, :])
        in_maps.append({"xT": xT, "wqk": wqk, "wv": wv, "wproj": wp,
                        "rhT": rhT, "rwT": rwT, "onehot": oh, "oh6": oh6})
    return in_maps


_NC_CACHE = {}


def kernel(x, Wqkv, Wproj, bproj, rel_pos_h, rel_pos_w):
    if "nc" not in _NC_CACHE:
        _NC_CACHE["nc"] = build_program()
    nc = _NC_CACHE["nc"]
    in_maps = shard_inputs(x, Wqkv, Wproj, rel_pos_h, rel_pos_w)
    res = run_bass_kernel_spmd(nc, in_maps, list(range(NCORES)))
    bproj = np.asarray(bproj, dtype=np.float32)
    out = np.empty((B, H, W, C), dtype=np.float32)
    for b in range(B):
        acc = (res.results[2 * b]["out"].astype(np.float32)
               + res.results[2 * b + 1]["out"].astype(np.float32) + bproj)
        out[b] = acc.reshape(H, W, C)
    return out


# revision 34
# speedup vs baseline: 1.1819x; 1.0469x over previous
"""Trainium2 Bass kernel for ViTDet-style attention with decomposed
relative-position bias.

Problem shapes (hardcoded):
  x: (4, 32, 32, 768) f32, Wqkv: (768, 2304), Wproj: (768, 768),
  bproj: (768,), rel_pos_h/w: (63, 64).
  12 heads, head_dim 64, S = 32*32 = 1024.

Sharding: 48 (batch, head) pairs -> 6 heads per core, all of one batch per
core-pair. Each core computes its heads' attention and a partial output
projection (its heads' channel rows of Wproj); the host sums the two
partials (bf16) per batch and adds bproj.

Device algorithm per core (bf16 matmuls, fp32 PSUM accumulation):
  - inputs land in a handful of large strided DMAs (the sync engine's
    per-DMA issue cost, ~0.7us, was serializing the load phase)
  - qkT = Wqk^T @ x^T  (x^T supplied pre-transposed by host; k pre-scaled)
  - v   = x @ Wv       (natural layout, with an appended ones column)
  - rel-pos bias computed directly per h/w block with the rel table slice
    as stationary and a 3D strided AP moving over all 6 heads at once;
    the w-axis is staged w-major (contiguous copies) and permuted once
    per head.
  - scoresT (k x q) = kaugT^T @ qaugT in ONE K=128 matmul per tile:
    aug rows 0-63 = kT / qT, 64-95 = one-hot(h) / BhT, 96-127 = one-hot(w)/BwT
    (one-hot rows DMA'd straight from DRAM into kaug).
  - eT = exp(scoresT): head A of each pair on ScalarE (table exp), head B
    on VectorE via the Schraudolph bit trick (i16 = round(a*x+b) viewed as
    bf16) so the two exps of a round run on different engines.
  - avT (65 x q) accumulates v_aug^T-matmul over k blocks; row 64 = softmax
    denominator via the ones column.  Head pairs are interleaved with av
    matmuls one k-block behind the score matmuls.
  - normalize on-chip: copy av to SBUF (frees PSUM fast), SBUF->SBUF DMA
    reshapes the denominator row to (128,8), DVE reciprocal + cast to bf16,
    DMA back to a row, gpsimd partition-broadcast (last pair: PE K=1
    broadcast, since the PE is idle there), DVE multiply.
  - partial = out_heads @ Wproj_shard, emitted bf16 to DRAM.
"""

import numpy as np

import concourse.bass as bass
import concourse.bacc as bacc
import concourse.mybir as mybir
import concourse.tile as tile
from concourse.bass_utils import run_bass_kernel_spmd

F32 = mybir.dt.float32
BF16 = mybir.dt.bfloat16
I16 = mybir.dt.int16

NH = 12          # total heads
C = 768
HD = 64
H = W = 32
S = H * W        # 1024
B = 4
NCORES = 8
HPC = NH * B // NCORES   # heads per core = 6
NCH = 6                  # C // 128 input-channel chunks
NKB = S // 128           # 8 k blocks
NQB = S // 128           # 8 q blocks
NHALF = 512              # matmul moving-dim half

# Schraudolph exp constants for bf16 output: bits = round(EXP_A*x + EXP_B)
EXP_A = 184.66502304     # 2^7 / ln 2
EXP_B = 16247.75


def build_program():
    nc = bacc.Bacc("TRN2", target_bir_lowering=False, debug=False)

    # inputs pre-chunked on the host so every DMA is a contiguous DRAM read
    xT = nc.declare_dram_parameter("xT", [128, NCH * S], BF16, isOutput=False)
    wqk = nc.declare_dram_parameter("wqk", [128, NCH * 2 * HPC * HD], BF16,
                                    isOutput=False)
    wv = nc.declare_dram_parameter("wv", [128, NCH * HPC * HD], BF16,
                                   isOutput=False)
    wproj = nc.declare_dram_parameter("wproj", [128, 3 * C], BF16, isOutput=False)
    rhT = nc.declare_dram_parameter("rhT", [HD, 2 * H - 1], BF16, isOutput=False)
    rwT = nc.declare_dram_parameter("rwT", [HD, 2 * W - 1], BF16, isOutput=False)
    onehot = nc.declare_dram_parameter("onehot", [65, S], BF16, isOutput=False)
    oh6 = nc.declare_dram_parameter("oh6", [64, HPC * S], BF16, isOutput=False)
    out = nc.declare_dram_parameter("out", [S, C], BF16, isOutput=True)

    with tile.TileContext(nc) as tc:
        with (
            tc.tile_pool(name="persist", bufs=1) as persist,
            tc.tile_pool(name="pa", bufs=2, space="PSUM") as pa,
            tc.tile_pool(name="pb", bufs=2, space="PSUM") as pb,
            tc.tile_pool(name="et", bufs=4) as et_pool,
            tc.tile_pool(name="small", bufs=2) as small,
        ):
            # all-heads augmented k/q tiles (128, 6*S)
            kaug = persist.tile([128, HPC * S], BF16, tag="kaug", name="kaug")
            qaug = persist.tile([128, HPC * S], BF16, tag="qaug", name="qaug")
            ones_sb = persist.tile([1, 64], BF16, tag="ones", name="ones_sb")
            nc.vector.memset(ones_sb[:], 1.0)

            # ---- persistent SBUF loads: few big strided DMAs ----
            xT_all = persist.tile([128, NCH * S], BF16, tag="xT", name="xT_all")
            wv_all = persist.tile([128, NCH * HPC * HD], BF16, tag="wv", name="wv_all")
            wqk_all = persist.tile([128, NCH * 2 * HPC * HD], BF16, tag="wqk",
                                   name="wqk_all")
            wproj_all = persist.tile([128, 3 * C], BF16, tag="wproj",
                                     name="wproj_all")
            # spread the loads across per-engine DMA queues so the
            # transfers run in parallel instead of serializing on one queue
            nc.sync.dma_start(xT_all[:, 0:3 * S], xT[:, 0:3 * S])
            nc.sync.dma_start(xT_all[:, 3 * S:6 * S], xT[:, 3 * S:6 * S])
            nc.scalar.dma_start(wqk_all[:, 0:3 * 768], wqk[:, 0:3 * 768])
            nc.scalar.dma_start(wv_all[:], wv[:, :])
            nc.gpsimd.dma_start(wqk_all[:, 3 * 768:], wqk[:, 3 * 768:])
            oh = persist.tile([65, S], BF16, tag="onehot", name="onehot")
            nc.sync.dma_start(oh[:], onehot[:, :])
            rhT_sb = persist.tile([HD, 2 * H - 1], BF16, tag="rhT", name="rhT_sb")
            nc.sync.dma_start(rhT_sb[:], rhT[:, :])
            rwT_sb = persist.tile([HD, 2 * W - 1], BF16, tag="rwT", name="rwT_sb")
            nc.sync.dma_start(rwT_sb[:], rwT[:, :])
            # off the critical path: issue from the (idle) gpsimd queue
            nc.gpsimd.dma_start(wproj_all[:], wproj[:, :])
            nc.gpsimd.dma_start(kaug[64:128, :], oh6[:, :])

            def xs(ci):
                return xT_all[:, S * ci:S * (ci + 1)]

            # ---- v projection (natural) + ones column ----
            # v_sb[sb]: (128, 6*65) cols [65i..65i+64) = head i v, col 65i+64 = 1
            # emitted interleaved into the qk/bias phase (v is first needed
            # only when the attention av units start)
            v_sb = [persist.tile([128, HPC * (HD + 1)], BF16, tag=f"v{sb}", name=f"v{sb}")
                    for sb in range(NKB)]

            def v_unit(sb):
                vp = pa.tile([128, HPC * HD + HPC], F32, tag="big", name="vp")
                for ci in range(NCH):
                    nc.tensor.matmul(
                        vp[:, 0:HPC * HD],
                        xs(ci)[:, 128 * sb:128 * (sb + 1)],
                        wv_all[:, HPC * HD * ci:HPC * HD * (ci + 1)],
                        start=(ci == 0), stop=(ci == NCH - 1))
                nc.tensor.matmul(vp[:, HPC * HD:HPC * HD + HPC],
                                 oh[64:65, 128 * sb:128 * (sb + 1)],
                                 oh[64:65, 0:HPC], start=True, stop=True)
                vdst = v_sb[sb].rearrange("p (i c) -> p i c", c=HD + 1)
                nc.vector.tensor_copy(
                    vdst[:, :, 0:HD],
                    vp[:, 0:HPC * HD].rearrange("p (i c) -> p i c", c=HD))
                nc.vector.tensor_copy(
                    vdst[:, :, HD:HD + 1],
                    vp[:, HPC * HD:HPC * HD + HPC].rearrange("p (i c) -> p i c", c=1))

            # ---- qk projection (transposed layout) ----
            # octile t covers oc rows [128t, 128t+128): t<3 -> q, t>=3 -> k
            def qk_octile(t):
                qp = pa.tile([128, S], F32, tag="big", name="qp")
                for ci in range(NCH):
                    for nh in range(S // NHALF):
                        nc.tensor.matmul(
                            qp[:, NHALF * nh:NHALF * (nh + 1)],
                            wqk_all[:, 768 * ci + 128 * t:768 * ci + 128 * (t + 1)],
                            xs(ci)[:, NHALF * nh:NHALF * (nh + 1)],
                            start=(ci == 0), stop=(ci == NCH - 1))
                for sub in range(2):
                    if t < 3:
                        head = 2 * t + sub
                        nc.scalar.copy(qaug[0:64, S * head:S * (head + 1)],
                                       qp[64 * sub:64 * sub + 64, :])
                    else:
                        head = 2 * (t - 3) + sub
                        nc.vector.tensor_copy(
                            kaug[0:64, S * head:S * (head + 1)],
                            qp[64 * sub:64 * sub + 64, :])

            for t in range(3):
                qk_octile(t)

            # ---- rel-pos bias directly into qaug rows 64:128 ----
            # h-axis: qaug[64+kh', S*i + 32*h0 + w] = sum_c rhT[c, h0+kh'] qT_i[c,(h0,w)]
            # w-axis: qaug[96+kw', S*i + 32*h + w0] = sum_c rwT[c, w0+kw'] qT_i[c,(h,w0)]
            q3h = qaug[0:64, :].rearrange("p (i x) -> p i x", i=HPC)
            q3w = qaug[0:64, :].rearrange("p (i h w) -> p i w h", i=HPC, h=H)
            bh3 = qaug[64:96, :].rearrange("p (i x) -> p i x", i=HPC)
            # w-axis results staged w-major (contiguous copies), then one
            # strided permute copy per head into qaug rows 96:128
            bw_stage = persist.tile([32, HPC * S], BF16, tag="bwst", name="bw_stage")
            bw3 = bw_stage.rearrange("p (i w h) -> p i w h", i=HPC, w=W)

            def bias_chunk(axis, trange):
                for t in trange:
                    bps = (pa if t % 2 else pb).tile(
                        [32, 2 * HPC * 32], F32,
                        tag="big" if t % 2 else "av", name="bps")
                    for j in range(2):
                        d0 = 2 * t + j
                        if axis == 0:
                            nc.tensor.matmul(
                                bps[:, 192 * j:192 * (j + 1)],
                                rhT_sb[:, d0:d0 + 32],
                                q3h[:, :, 32 * d0:32 * (d0 + 1)],
                                start=True, stop=True)
                        else:
                            nc.tensor.matmul(
                                bps[:, 192 * j:192 * (j + 1)],
                                rwT_sb[:, d0:d0 + 32],
                                q3w[:, :, d0, :],
                                start=True, stop=True)
                    for j in range(2):
                        d0 = 2 * t + j
                        src = bps[:, 192 * j:192 * (j + 1)].rearrange(
                            "p (i x) -> p i x", i=HPC)
                        dst = bh3[:, :, 32 * d0:32 * (d0 + 1)] if axis == 0 \
                            else bw3[:, :, d0, :]
                        (nc.scalar.copy if j == 0 else nc.vector.tensor_copy)(
                            dst, src)

            # interleave the k octiles, v units, and bias chunks: the v
            # matmuls fill the PE while the bias copies drain
            qk_octile(3)
            v_unit(0)
            v_unit(1)
            bias_chunk(0, range(0, 8))
            qk_octile(4)
            v_unit(2)
            v_unit(3)
            bias_chunk(0, range(8, 16))
            qk_octile(5)
            v_unit(4)
            v_unit(5)
            bias_chunk(1, range(0, 8))
            v_unit(6)
            v_unit(7)
            bias_chunk(1, range(8, 16))
            # permute the staged w-axis bias (w-major -> h-major) per head
            for i in range(HPC):
                src = bw_stage[:, S * i:S * (i + 1)].rearrange(
                    "p (w h) -> p h w", w=W)
                (nc.scalar.copy if i % 2 else nc.vector.tensor_copy)(
                    qaug[96:128, S * i:S * (i + 1)], src)
            # bridge the copy-drain seam with PE work so the clock gate
            # stays open into the attention phase
            for _ in range(6):
                wp_ = pa.tile([128, NHALF], F32, tag="big", name="wp_")
                nc.tensor.matmul(wp_[:], kaug[:, 0:128], kaug[:, 0:NHALF],
                                 start=True, stop=True)

            # ---- attention: heads in interleaved pairs ----
            # av matmuls lag the score matmuls by TWO k-blocks so they never
            # wait on an exp, and each pair's two trailing av rounds overlap
            # the next pair's first two score rounds: the PE stream across
            # the whole attention phase has no dependency bubbles (keeps the
            # HAM clock gate at 2.4 GHz).
            out_headsT = [persist.tile([128, S], BF16, tag=f"ohT{c}",
                                       name=f"ohT{c}")
                          for c in range(HPC * HD // 128)]
            npairs = HPC // 2
            e_tiles = {}
            av_tiles = {}

            def sc_unit(pair, r):
                for i in (2 * pair, 2 * pair + 1):
                    sc = pa.tile([128, S], F32, tag="big", name="sc")
                    for nh in range(S // NHALF):
                        sl = slice(NHALF * nh, NHALF * (nh + 1))
                        nc.tensor.matmul(
                            sc[:, sl],
                            kaug[:, S * i + 128 * r:S * i + 128 * (r + 1)],
                            qaug[:, S * i + NHALF * nh:S * i + NHALF * (nh + 1)],
                            start=True, stop=True)
                    if i == 2 * pair:
                        e = et_pool.tile([128, S], BF16, tag="eta", name="eta",
                                         bufs=3)
                        nc.scalar.activation(
                            e[:], sc[:], mybir.ActivationFunctionType.Exp)
                        e_tiles[(i, r)] = e[:]
                    else:
                        ei = et_pool.tile([128, S], I16, tag="etb", name="etb",
                                          bufs=3)
                        nc.vector.tensor_scalar(
                            ei[:], sc[:], EXP_A, EXP_B,
                            op0=mybir.AluOpType.mult,
                            op1=mybir.AluOpType.add)
                        e_tiles[(i, r)] = ei[:].bitcast(BF16)

            def av_unit(pair, r):
                for i in (2 * pair, 2 * pair + 1):
                    if r == 0:
                        av_tiles[i] = pb.tile([HD + 1, S], F32, tag="av",
                                              name=f"av{i}")
                    e = e_tiles.pop((i, r))
                    for nh in range(S // NHALF):
                        sl = slice(NHALF * nh, NHALF * (nh + 1))
                        nc.tensor.matmul(
                            av_tiles[i][:, sl],
                            v_sb[r][:, (HD + 1) * i:(HD + 1) * (i + 1)],
                            e[:, sl],
                            start=(r == 0), stop=(r == NKB - 1))

            def norm(pair):
                # normalize: free PSUM fast via an SBUF copy, reshape the
                # denominator row across partitions by DMA, reciprocal,
                # broadcast (gpsimd; PE K=1 matmul for the last pair, when
                # the PE is otherwise idle), multiply.
                last = pair == npairs - 1
                for i in (2 * pair, 2 * pair + 1):
                    av = av_tiles.pop(i)
                    av_sb = small.tile([HD + 1, S], F32, tag="av_sb", name="av_sb")
                    nc.scalar.copy(av_sb[:], av[:])
                    den_t = small.tile([128, NQB], F32, tag="den_t", name="den_t")
                    nc.sync.dma_start(den_t[:], av_sb[HD:HD + 1, :])
                    rec_t = small.tile([128, NQB], F32, tag="rec_t", name="rec_t")
                    nc.vector.reciprocal(rec_t[:], den_t[:])
                    rec_b = small.tile([128, NQB], BF16, tag="rec_b", name="rec_b")
                    nc.vector.tensor_copy(rec_b[:], rec_t[:])
                    den_row = small.tile([1, S], BF16, tag="den_row", name="den_row")
                    nc.sync.dma_start(den_row[:], rec_b[:])
                    chunk, row = i // 2, (i % 2) * 64
                    if last:
                        rbp = pa.tile([64, S], F32, tag="big", name="rbp")
                        for nh in range(S // NHALF):
                            sl = slice(NHALF * nh, NHALF * (nh + 1))
                            nc.tensor.matmul(rbp[:, sl], ones_sb[:],
                                             den_row[:, sl], start=True, stop=True)
                        nc.vector.tensor_tensor(
                            out_headsT[chunk][row:row + 64, :], av_sb[0:HD, :],
                            rbp[:], op=mybir.AluOpType.mult)
                    else:
                        rb = small.tile([64, S], BF16, tag="rbcast", name="rbcast")
                        nc.gpsimd.partition_broadcast(rb[:], den_row[:])
                        nc.vector.tensor_tensor(
                            out_headsT[chunk][row:row + 64, :], av_sb[0:HD, :],
                            rb[:], op=mybir.AluOpType.mult)

            for s in range(8 * npairs + 2):
                for pair in range(npairs):
                    if 0 <= s - 8 * pair <= 7:
                        sc_unit(pair, s - 8 * pair)
                for pair in range(npairs):
                    r_av = s - 8 * pair - 2
                    if 0 <= r_av <= 7:
                        av_unit(pair, r_av)
                        if r_av == 7:
                            norm(pair)

            # keep the PE clock warm while the last pair's normalization
            # chain drains (these overlap it, they don't delay anything)
            scratch = persist.tile([128, NHALF], BF16, tag="scratch", name="scratch")
            for _ in range(10):
                wp_ = pa.tile([128, NHALF], F32, tag="big", name="wp_")
                nc.tensor.matmul(wp_[:], kaug[:, 0:128], kaug[:, 0:NHALF],
                                 start=True, stop=True)

            # ---- output projection (partial) ----
            for qb in range(NQB):
                pp = pa.tile([128, S], F32, tag="big", name="pp")
                for ci in range(HPC * HD // 128):
                    nc.tensor.matmul(
                        pp[:, 0:NHALF],
                        out_headsT[ci][:, 128 * qb:128 * (qb + 1)],
                        wproj_all[:, C * ci:C * ci + NHALF],
                        start=(ci == 0), stop=(ci == 2))
                    nc.tensor.matmul(
                        pp[:, NHALF:C],
                        out_headsT[ci][:, 128 * qb:128 * (qb + 1)],
                        wproj_all[:, C * ci + NHALF:C * (ci + 1)],
                        start=(ci == 0), stop=(ci == 2))
                pp_sb = small.tile([128, C], BF16, tag="pp_sb", name="pp_sb")
                (nc.scalar.copy if qb % 2 else nc.vector.tensor_copy)(
                    pp_sb[:], pp[:, 0:C])
                (nc.sync if qb % 2 else nc.scalar).dma_start(
                    out[128 * qb:128 * (qb + 1), :], pp_sb[:])

    nc.compile()
    return nc


def shard_inputs(x, Wqkv, Wproj, rel_pos_h, rel_pos_w):
    """Build the 8 per-core input maps."""
    import ml_dtypes
    bf16 = ml_dtypes.bfloat16
    scale = HD ** (-0.5)
    x = np.asarray(x, dtype=np.float32)
    Wqkv = np.asarray(Wqkv, dtype=np.float32)
    Wproj = np.asarray(Wproj, dtype=np.float32)
    rhT = np.ascontiguousarray(np.asarray(rel_pos_h, np.float32).T).astype(bf16)
    rwT = np.ascontiguousarray(np.asarray(rel_pos_w, np.float32).T).astype(bf16)
    oh = np.zeros((65, S), np.float32)
    for khp in range(H):
        oh[khp, (31 - khp) * W:(31 - khp) * W + W] = 1.0
    for kwp in range(W):
        oh[32 + kwp, 31 - kwp::W] = 1.0
    oh[64, :] = 1.0
    oh = oh.astype(bf16)
    oh6 = np.ascontiguousarray(np.tile(oh[0:64, :], (1, HPC)))

    def chunk(a):
        # (n*128, m) -> (128, n*m) with 128-row chunks side by side
        n = a.shape[0] // 128
        return np.ascontiguousarray(
            a.reshape(n, 128, a.shape[1]).transpose(1, 0, 2).reshape(
                128, n * a.shape[1])).astype(bf16)

    in_maps = []
    for core in range(NCORES):
        b = core // 2
        h0 = (core % 2) * HPC
        xb = x[b].reshape(S, C)
        xT = chunk(np.ascontiguousarray(xb.T))
        wq = Wqkv[:, h0 * HD:(h0 + HPC) * HD]
        wk = Wqkv[:, C + h0 * HD:C + (h0 + HPC) * HD] * scale
        wqk = chunk(np.concatenate([wq, wk], axis=1))
        wv = chunk(Wqkv[:, 2 * C + h0 * HD:2 * C + (h0 + HPC) * HD])
        wp = chunk(Wproj[h0 * HD:(h0 + HPC) * HD, :])
        in_maps.append({"xT": xT, "wqk": wqk, "wv": wv, "wproj": wp,
                        "rhT": rhT, "rwT": rwT, "onehot": oh, "oh6": oh6})
    return in_maps


_NC_CACHE = {}


def kernel(x, Wqkv, Wproj, bproj, rel_pos_h, rel_pos_w):
    if "nc" not in _NC_CACHE:
        _NC_CACHE["nc"] = build_program()
    nc = _NC_CACHE["nc"]
    in_maps = shard_inputs(x, Wqkv, Wproj, rel_pos_h, rel_pos_w)
    res = run_bass_kernel_spmd(nc, in_maps, list(range(NCORES)))
    bproj = np.asarray(bproj, dtype=np.float32)
    out = np.empty((B, H, W, C), dtype=np.float32)
    for b in range(B):
        acc = (res.results[2 * b]["out"].astype(np.float32)
               + res.results[2 * b + 1]["out"].astype(np.float32) + bproj)
        out[b] = acc.reshape(H, W, C)
    return out


# revision 35
# speedup vs baseline: 1.1995x; 1.0149x over previous
"""Trainium2 Bass kernel for ViTDet-style attention with decomposed
relative-position bias.

Problem shapes (hardcoded):
  x: (4, 32, 32, 768) f32, Wqkv: (768, 2304), Wproj: (768, 768),
  bproj: (768,), rel_pos_h/w: (63, 64).
  12 heads, head_dim 64, S = 32*32 = 1024.

Sharding: 48 (batch, head) pairs -> 6 heads per core, all of one batch per
core-pair. Each core computes its heads' attention and a partial output
projection (its heads' channel rows of Wproj); the host sums the two
partials (bf16) per batch and adds bproj.

Device algorithm per core (bf16 matmuls, fp32 PSUM accumulation):
  - inputs land in a handful of large strided DMAs (the sync engine's
    per-DMA issue cost, ~0.7us, was serializing the load phase)
  - qkT = Wqk^T @ x^T  (x^T supplied pre-transposed by host; k pre-scaled)
  - v   = x @ Wv       (natural layout, with an appended ones column)
  - rel-pos bias computed directly per h/w block with the rel table slice
    as stationary and a 3D strided AP moving over all 6 heads at once;
    the w-axis is staged w-major (contiguous copies) and permuted once
    per head.
  - scoresT (k x q) = kaugT^T @ qaugT in ONE K=128 matmul per tile:
    aug rows 0-63 = kT / qT, 64-95 = one-hot(h) / BhT, 96-127 = one-hot(w)/BwT
    (one-hot rows DMA'd straight from DRAM into kaug).
  - eT = exp(scoresT): head A of each pair on ScalarE (table exp), head B
    on VectorE via the Schraudolph bit trick (i16 = round(a*x+b) viewed as
    bf16) so the two exps of a round run on different engines.
  - avT (65 x q) accumulates v_aug^T-matmul over k blocks; row 64 = softmax
    denominator via the ones column.  Head pairs are interleaved with av
    matmuls one k-block behind the score matmuls.
  - normalize on-chip: copy av to SBUF (frees PSUM fast), SBUF->SBUF DMA
    reshapes the denominator row to (128,8), DVE reciprocal + cast to bf16,
    DMA back to a row, gpsimd partition-broadcast (last pair: PE K=1
    broadcast, since the PE is idle there), DVE multiply.
  - partial = out_heads @ Wproj_shard, emitted bf16 to DRAM.
"""

import numpy as np

import concourse.bass as bass
import concourse.bacc as bacc
import concourse.mybir as mybir
import concourse.tile as tile
from concourse.bass_utils import run_bass_kernel_spmd

F32 = mybir.dt.float32
BF16 = mybir.dt.bfloat16
I16 = mybir.dt.int16

NH = 12          # total heads
C = 768
HD = 64
H = W = 32
S = H * W        # 1024
B = 4
NCORES = 8
HPC = NH * B // NCORES   # heads per core = 6
NCH = 6                  # C // 128 input-channel chunks
NKB = S // 128           # 8 k blocks
NQB = S // 128           # 8 q blocks
NHALF = 512              # matmul moving-dim half

# Schraudolph exp constants for bf16 output: bits = round(EXP_A*x + EXP_B)
EXP_A = 184.66502304     # 2^7 / ln 2
EXP_B = 16247.75


def build_program():
    nc = bacc.Bacc("TRN2", target_bir_lowering=False, debug=False)

    # inputs pre-chunked on the host so every DMA is a contiguous DRAM read
    xT = nc.declare_dram_parameter("xT", [128, NCH * S], BF16, isOutput=False)
    wqk = nc.declare_dram_parameter("wqk", [128, NCH * 2 * HPC * HD], BF16,
                                    isOutput=False)
    wv = nc.declare_dram_parameter("wv", [128, NCH * HPC * HD], BF16,
                                   isOutput=False)
    wproj = nc.declare_dram_parameter("wproj", [128, 3 * C], BF16, isOutput=False)
    rhT = nc.declare_dram_parameter("rhT", [HD, 2 * H - 1], BF16, isOutput=False)
    rwT = nc.declare_dram_parameter("rwT", [HD, 2 * W - 1], BF16, isOutput=False)
    onehot = nc.declare_dram_parameter("onehot", [65, S], BF16, isOutput=False)
    oh6 = nc.declare_dram_parameter("oh6", [64, HPC * S], BF16, isOutput=False)
    out = nc.declare_dram_parameter("out", [S, C], BF16, isOutput=True)

    with tile.TileContext(nc) as tc:
        with (
            tc.tile_pool(name="persist", bufs=1) as persist,
            tc.tile_pool(name="pa", bufs=2, space="PSUM") as pa,
            tc.tile_pool(name="pb", bufs=2, space="PSUM") as pb,
            tc.tile_pool(name="et", bufs=4) as et_pool,
            tc.tile_pool(name="small", bufs=2) as small,
        ):
            # all-heads augmented k/q tiles (128, 6*S)
            kaug = persist.tile([128, HPC * S], BF16, tag="kaug", name="kaug")
            qaug = persist.tile([128, HPC * S], BF16, tag="qaug", name="qaug")
            ones_sb = persist.tile([1, 64], BF16, tag="ones", name="ones_sb")
            nc.vector.memset(ones_sb[:], 1.0)

            # ---- persistent SBUF loads: few big strided DMAs ----
            xT_all = persist.tile([128, NCH * S], BF16, tag="xT", name="xT_all")
            wv_all = persist.tile([128, NCH * HPC * HD], BF16, tag="wv", name="wv_all")
            wqk_all = persist.tile([128, NCH * 2 * HPC * HD], BF16, tag="wqk",
                                   name="wqk_all")
            wproj_all = persist.tile([128, 3 * C], BF16, tag="wproj",
                                     name="wproj_all")
            # spread the loads across per-engine DMA queues so the
            # transfers run in parallel instead of serializing on one queue
            nc.sync.dma_start(xT_all[:, 0:3 * S], xT[:, 0:3 * S])
            nc.sync.dma_start(xT_all[:, 3 * S:6 * S], xT[:, 3 * S:6 * S])
            nc.scalar.dma_start(wqk_all[:, 0:3 * 768], wqk[:, 0:3 * 768])
            nc.scalar.dma_start(wv_all[:], wv[:, :])
            nc.gpsimd.dma_start(wqk_all[:, 3 * 768:], wqk[:, 3 * 768:])
            oh = persist.tile([65, S], BF16, tag="onehot", name="onehot")
            nc.sync.dma_start(oh[:], onehot[:, :])
            rhT_sb = persist.tile([HD, 2 * H - 1], BF16, tag="rhT", name="rhT_sb")
            nc.sync.dma_start(rhT_sb[:], rhT[:, :])
            rwT_sb = persist.tile([HD, 2 * W - 1], BF16, tag="rwT", name="rwT_sb")
            nc.sync.dma_start(rwT_sb[:], rwT[:, :])
            # off the critical path: issue from the (idle) gpsimd queue
            nc.gpsimd.dma_start(wproj_all[:], wproj[:, :])
            nc.gpsimd.dma_start(kaug[64:128, :], oh6[:, :])

            def xs(ci):
                return xT_all[:, S * ci:S * (ci + 1)]

            # ---- v projection (natural) + ones column ----
            # v_sb[sb]: (128, 6*65) cols [65i..65i+64) = head i v, col 65i+64 = 1
            # emitted interleaved into the qk/bias phase (v is first needed
            # only when the attention av units start)
            v_sb = [persist.tile([128, HPC * (HD + 1)], BF16, tag=f"v{sb}", name=f"v{sb}")
                    for sb in range(NKB)]

            def v_unit(sb):
                vp = pa.tile([128, HPC * HD + HPC], F32, tag="big", name="vp")
                for ci in range(NCH):
                    nc.tensor.matmul(
                        vp[:, 0:HPC * HD],
                        xs(ci)[:, 128 * sb:128 * (sb + 1)],
                        wv_all[:, HPC * HD * ci:HPC * HD * (ci + 1)],
                        start=(ci == 0), stop=(ci == NCH - 1))
                nc.tensor.matmul(vp[:, HPC * HD:HPC * HD + HPC],
                                 oh[64:65, 128 * sb:128 * (sb + 1)],
                                 oh[64:65, 0:HPC], start=True, stop=True)
                vdst = v_sb[sb].rearrange("p (i c) -> p i c", c=HD + 1)
                nc.vector.tensor_copy(
                    vdst[:, :, 0:HD],
                    vp[:, 0:HPC * HD].rearrange("p (i c) -> p i c", c=HD))
                nc.vector.tensor_copy(
                    vdst[:, :, HD:HD + 1],
                    vp[:, HPC * HD:HPC * HD + HPC].rearrange("p (i c) -> p i c", c=1))

            # ---- qk projection (transposed layout) ----
            # octile t covers oc rows [128t, 128t+128): t<3 -> q, t>=3 -> k
            def qk_octile(t):
                qp = pa.tile([128, S], F32, tag="big", name="qp")
                for ci in range(NCH):
                    for nh in range(S // NHALF):
                        nc.tensor.matmul(
                            qp[:, NHALF * nh:NHALF * (nh + 1)],
                            wqk_all[:, 768 * ci + 128 * t:768 * ci + 128 * (t + 1)],
                            xs(ci)[:, NHALF * nh:NHALF * (nh + 1)],
                            start=(ci == 0), stop=(ci == NCH - 1))
                for sub in range(2):
                    if t < 3:
                        head = 2 * t + sub
                        nc.scalar.copy(qaug[0:64, S * head:S * (head + 1)],
                                       qp[64 * sub:64 * sub + 64, :])
                    else:
                        head = 2 * (t - 3) + sub
                        nc.vector.tensor_copy(
                            kaug[0:64, S * head:S * (head + 1)],
                            qp[64 * sub:64 * sub + 64, :])

            for t in range(3):
                qk_octile(t)

            # ---- rel-pos bias directly into qaug rows 64:128 ----
            # h-axis: qaug[64+kh', S*i + 32*h0 + w] = sum_c rhT[c, h0+kh'] qT_i[c,(h0,w)]
            # w-axis: qaug[96+kw', S*i + 32*h + w0] = sum_c rwT[c, w0+kw'] qT_i[c,(h,w0)]
            q3h = qaug[0:64, :].rearrange("p (i x) -> p i x", i=HPC)
            q3w = qaug[0:64, :].rearrange("p (i h w) -> p i w h", i=HPC, h=H)
            bh3 = qaug[64:96, :].rearrange("p (i x) -> p i x", i=HPC)
            # w-axis results staged w-major (contiguous copies), then one
            # strided permute copy per head into qaug rows 96:128
            bw_stage = persist.tile([32, HPC * S], BF16, tag="bwst", name="bw_stage")
            bw3 = bw_stage.rearrange("p (i w h) -> p i w h", i=HPC, w=W)

            def bias_chunk(axis, trange):
                for t in trange:
                    bps = (pa if t % 2 else pb).tile(
                        [32, 2 * HPC * 32], F32,
                        tag="big" if t % 2 else "av", name="bps")
                    for j in range(2):
                        d0 = 2 * t + j
                        if axis == 0:
                            nc.tensor.matmul(
                                bps[:, 192 * j:192 * (j + 1)],
                                rhT_sb[:, d0:d0 + 32],
                                q3h[:, :, 32 * d0:32 * (d0 + 1)],
                                start=True, stop=True)
                        else:
                            nc.tensor.matmul(
                                bps[:, 192 * j:192 * (j + 1)],
                                rwT_sb[:, d0:d0 + 32],
                                q3w[:, :, d0, :],
                                start=True, stop=True)
                    for j in range(2):
                        d0 = 2 * t + j
                        src = bps[:, 192 * j:192 * (j + 1)].rearrange(
                            "p (i x) -> p i x", i=HPC)
                        dst = bh3[:, :, 32 * d0:32 * (d0 + 1)] if axis == 0 \
                            else bw3[:, :, d0, :]
                        (nc.scalar.copy if j == 0 else nc.vector.tensor_copy)(
                            dst, src)

            # interleave the k octiles, v units, and bias chunks: the v
            # matmuls fill the PE while the bias copies drain
            qk_octile(3)
            v_unit(0)
            v_unit(1)
            bias_chunk(0, range(0, 8))
            qk_octile(4)
            v_unit(2)
            v_unit(3)
            bias_chunk(0, range(8, 16))
            qk_octile(5)
            v_unit(4)
            v_unit(5)
            bias_chunk(1, range(0, 8))
            v_unit(6)
            v_unit(7)
            bias_chunk(1, range(8, 16))
            # permute the staged w-axis bias (w-major -> h-major) per head
            for i in range(HPC):
                src = bw_stage[:, S * i:S * (i + 1)].rearrange(
                    "p (w h) -> p h w", w=W)
                (nc.scalar.copy if i % 2 else nc.vector.tensor_copy)(
                    qaug[96:128, S * i:S * (i + 1)], src)
            # bridge the copy-drain seam with PE work so the clock gate
            # stays open into the attention phase
            for _ in range(6):
                wp_ = pa.tile([128, NHALF], F32, tag="big", name="wp_")
                nc.tensor.matmul(wp_[:], kaug[:, 0:128], kaug[:, 0:NHALF],
                                 start=True, stop=True)

            # ---- attention: heads in interleaved pairs ----
            # av matmuls lag the score matmuls by TWO k-blocks so they never
            # wait on an exp, and each pair's two trailing av rounds overlap
            # the next pair's first two score rounds: the PE stream across
            # the whole attention phase has no dependency bubbles (keeps the
            # HAM clock gate at 2.4 GHz).
            out_headsT = [persist.tile([128, S], BF16, tag=f"ohT{c}",
                                       name=f"ohT{c}")
                          for c in range(HPC * HD // 128)]
            npairs = HPC // 2
            e_tiles = {}
            av_tiles = {}

            def sc_unit(pair, r):
                for i in (2 * pair, 2 * pair + 1):
                    sc = pa.tile([128, S], F32, tag="big", name="sc")
                    for nh in range(S // NHALF):
                        sl = slice(NHALF * nh, NHALF * (nh + 1))
                        nc.tensor.matmul(
                            sc[:, sl],
                            kaug[:, S * i + 128 * r:S * i + 128 * (r + 1)],
                            qaug[:, S * i + NHALF * nh:S * i + NHALF * (nh + 1)],
                            start=True, stop=True)
                    if i == 2 * pair:
                        e = et_pool.tile([128, S], BF16, tag="eta", name="eta",
                                         bufs=3)
                        nc.scalar.activation(
                            e[:], sc[:], mybir.ActivationFunctionType.Exp)
                        e_tiles[(i, r)] = e[:]
                    else:
                        ei = et_pool.tile([128, S], I16, tag="etb", name="etb",
                                          bufs=3)
                        nc.vector.tensor_scalar(
                            ei[:], sc[:], EXP_A, EXP_B,
                            op0=mybir.AluOpType.mult,
                            op1=mybir.AluOpType.add)
                        e_tiles[(i, r)] = ei[:].bitcast(BF16)

            def av_unit(pair, r):
                for i in (2 * pair, 2 * pair + 1):
                    if r == 0:
                        av_tiles[i] = pb.tile([HD + 1, S], F32, tag="av",
                                              name=f"av{i}")
                    e = e_tiles.pop((i, r))
                    for nh in range(S // NHALF):
                        sl = slice(NHALF * nh, NHALF * (nh + 1))
                        nc.tensor.matmul(
                            av_tiles[i][:, sl],
                            v_sb[r][:, (HD + 1) * i:(HD + 1) * (i + 1)],
                            e[:, sl],
                            start=(r == 0), stop=(r == NKB - 1))

            def norm(pair):
                # normalize: free PSUM fast via an SBUF copy, reshape the
                # denominator row across partitions by DMA, reciprocal,
                # broadcast (gpsimd; PE K=1 matmul for the last pair, when
                # the PE is otherwise idle), multiply.
                last = pair == npairs - 1
                for i in (2 * pair, 2 * pair + 1):
                    av = av_tiles.pop(i)
                    av_sb = small.tile([HD + 1, S], F32, tag="av_sb", name="av_sb")
                    nc.scalar.copy(av_sb[:], av[:])
                    den_t = small.tile([128, NQB], F32, tag="den_t", name="den_t")
                    nc.sync.dma_start(den_t[:], av_sb[HD:HD + 1, :])
                    rec_t = small.tile([128, NQB], F32, tag="rec_t", name="rec_t")
                    nc.vector.reciprocal(rec_t[:], den_t[:])
                    rec_b = small.tile([128, NQB], BF16, tag="rec_b", name="rec_b")
                    nc.vector.tensor_copy(rec_b[:], rec_t[:])
                    den_row = small.tile([1, S], BF16, tag="den_row", name="den_row")
                    nc.sync.dma_start(den_row[:], rec_b[:])
                    chunk, row = i // 2, (i % 2) * 64
                    if last:
                        rbp = pa.tile([64, S], F32, tag="big", name="rbp")
                        for nh in range(S // NHALF):
                            sl = slice(NHALF * nh, NHALF * (nh + 1))
                            nc.tensor.matmul(rbp[:, sl], ones_sb[:],
                                             den_row[:, sl], start=True, stop=True)
                        nc.vector.tensor_tensor(
                            out_headsT[chunk][row:row + 64, :], av_sb[0:HD, :],
                            rbp[:], op=mybir.AluOpType.mult)
                    else:
                        rb = small.tile([64, S], BF16, tag="rbcast", name="rbcast")
                        nc.gpsimd.partition_broadcast(rb[:], den_row[:])
                        nc.vector.tensor_tensor(
                            out_headsT[chunk][row:row + 64, :], av_sb[0:HD, :],
                            rb[:], op=mybir.AluOpType.mult)

            for s in range(8 * npairs + 2):
                for pair in range(npairs):
                    if 0 <= s - 8 * pair <= 7:
                        sc_unit(pair, s - 8 * pair)
                for pair in range(npairs):
                    r_av = s - 8 * pair - 2
                    if 0 <= r_av <= 7:
                        av_unit(pair, r_av)
                        if r_av == 7:
                            norm(pair)

            # keep the PE clock warm while the last pair's normalization
            # chain drains (these overlap it, they don't delay anything)
            scratch = persist.tile([128, NHALF], BF16, tag="scratch", name="scratch")
            for _ in range(10):
                wp_ = pa.tile([128, NHALF], F32, tag="big", name="wp_")
                nc.tensor.matmul(wp_[:], kaug[:, 0:128], kaug[:, 0:NHALF],
                                 start=True, stop=True)

            # ---- output projection (partial) ----
            # emitted per PAIR of query blocks with the ci=2 matmuls (the
            # ones gated by the last heads' normalization) trailing the
            # ready ci=0/1 work, so the PE FIFO isn't blocked at the seam
            def proj_mms(pp, qb, cis):
                for ci in cis:
                    nc.tensor.matmul(
                        pp[:, 0:NHALF],
                        out_headsT[ci][:, 128 * qb:128 * (qb + 1)],
                        wproj_all[:, C * ci:C * ci + NHALF],
                        start=(ci == 0), stop=(ci == 2))
                    nc.tensor.matmul(
                        pp[:, NHALF:C],
                        out_headsT[ci][:, 128 * qb:128 * (qb + 1)],
                        wproj_all[:, C * ci + NHALF:C * (ci + 1)],
                        start=(ci == 0), stop=(ci == 2))

            def proj_tail(pp, qb):
                proj_mms(pp, qb, (2,))
                pp_sb = small.tile([128, C], BF16, tag="pp_sb", name="pp_sb")
                (nc.scalar.copy if qb % 2 else nc.vector.tensor_copy)(
                    pp_sb[:], pp[:, 0:C])
                (nc.sync if qb % 2 else nc.scalar).dma_start(
                    out[128 * qb:128 * (qb + 1), :], pp_sb[:])

            for qb in range(0, NQB, 2):
                ppA = pa.tile([128, S], F32, tag="big", name="ppA")
                proj_mms(ppA, qb, (0, 1))
                ppB = pa.tile([128, S], F32, tag="big", name="ppB")
                proj_mms(ppB, qb + 1, (0, 1))
                proj_tail(ppA, qb)
                proj_tail(ppB, qb + 1)

    nc.compile()
    return nc


def shard_inputs(x, Wqkv, Wproj, rel_pos_h, rel_pos_w):
    """Build the 8 per-core input maps."""
    import ml_dtypes
    bf16 = ml_dtypes.bfloat16
    scale = HD ** (-0.5)
    x = np.asarray(x, dtype=np.float32)
    Wqkv = np.asarray(Wqkv, dtype=np.float32)
    Wproj = np.asarray(Wproj, dtype=np.float32)
    rhT = np.ascontiguousarray(np.asarray(rel_pos_h, np.float32).T).astype(bf16)
    rwT = np.ascontiguousarray(np.asarray(rel_pos_w, np.float32).T).astype(bf16)
    oh = np.zeros((65, S), np.float32)
    for khp in range(H):
        oh[khp, (31 - khp) * W:(31 - khp) * W + W] = 1.0
    for kwp in range(W):
        oh[32 + kwp, 31 - kwp::W] = 1.0
    oh[64, :] = 1.0
    oh = oh.astype(bf16)
    oh6 = np.ascontiguousarray(np.tile(oh[0:64, :], (1, HPC)))

    def chunk(a):
        # (n*128, m) -> (128, n*m) with 128-row chunks side by side
        n = a.shape[0] // 128
        return np.ascontiguousarray(
            a.reshape(n, 128, a.shape[1]).transpose(1, 0, 2).reshape(
                128, n * a.shape[1])).astype(bf16)

    in_maps = []
    for core in range(NCORES):
        b = core // 2
        h0 = (core % 2) * HPC
        xb = x[b].reshape(S, C)
        xT = chunk(np.ascontiguousarray(xb.T))
        wq = Wqkv[:, h0 * HD:(h0 + HPC) * HD]
        wk = Wqkv[:, C + h0 * HD:C + (h0 + HPC) * HD] * scale
        wqk = chunk(np.concatenate([wq, wk], axis=1))
        wv = chunk(Wqkv[:, 2 * C + h0 * HD:2 * C + (h0 + HPC) * HD])
        wp = chunk(Wproj[h0 * HD:(h0 + HPC) * HD, :])
        in_maps.append({"xT": xT, "wqk": wqk, "wv": wv, "wproj": wp,
                        "rhT": rhT, "rwT": rwT, "onehot": oh, "oh6": oh6})
    return in_maps


_NC_CACHE = {}


def kernel(x, Wqkv, Wproj, bproj, rel_pos_h, rel_pos_w):
    if "nc" not in _NC_CACHE:
        _NC_CACHE["nc"] = build_program()
    nc = _NC_CACHE["nc"]
    in_maps = shard_inputs(x, Wqkv, Wproj, rel_pos_h, rel_pos_w)
    res = run_bass_kernel_spmd(nc, in_maps, list(range(NCORES)))
    bproj = np.asarray(bproj, dtype=np.float32)
    out = np.empty((B, H, W, C), dtype=np.float32)
    for b in range(B):
        acc = (res.results[2 * b]["out"].astype(np.float32)
               + res.results[2 * b + 1]["out"].astype(np.float32) + bproj)
        out[b] = acc.reshape(H, W, C)
    return out
